# revision 1
# baseline (speedup 1.0000x reference)
"""DKVMN kernel for Trainium2 (8 NeuronCores, data-parallel over batch).

Shapes (hardcoded): B=64, S=200, INUM=1000, IN_DIM=2000, CNUM=50, EDIM=128.

Decomposition per core (B_loc = 8 batches, BT = B_loc*S = 1600 flat steps):
  Phase 1 (all timesteps, batched matmuls):
      itmT [e,bt]  = A_w @ item.T       (PE, contraction over INUM via PE-transposed item tiles)
      itrT [e,bt]  = B_w @ inter.T
      eT   [e,bt]  = sigmoid(er_w @ itrT + er_b)
      aT   [e,bt]  = tanh(ad_w @ itrT + ad_b)
      w    [c,bt]  = softmax_c(kmat @ itmT)   (exp on ACT, sum/broadcast via ones-matmuls)
  Phase 2 (sequential scan over S=200; only r depends on the recurrence):
      layout: V [e=128 part, (b=8, c=50) free]
      per step: Wbc = ones x w_t (PE K=1 matmul), then 5 DVE passes:
        X = V*Wbc ; r_t = reduce_c(X) ; P = X*e_bv ; Y = V - P ; V = Y + Wbc*a_bv
  Phase 3: hT = tanh(lin_w1 @ rT + lin_w2 @ itmT + lin_b);
           out = sigmoid(h @ cls_w.T + cls_b)  (natural [bt, inum] layout)
"""

import numpy as np

import concourse.bass as bass
import concourse.mybir as mybir
import concourse.tile as tile
from concourse import bacc
from concourse.bass_utils import run_bass_kernel_spmd
from concourse.masks import make_identity

F32 = mybir.dt.float32
AF = mybir.ActivationFunctionType
OP = mybir.AluOpType

B, S, INUM, IN_DIM, CNUM, EDIM = 64, 200, 1000, 2000, 50, 128
NCORES = 8
BL = B // NCORES          # 8 batches per core
BT = BL * S               # 1600
IK = 8                    # INUM k-chunks of 125
DK = 16                   # IN_DIM k-chunks of 125
KC = 125                  # k-chunk width

_NC_CACHE = {}
LAST_RESULT = None


def _build():
    nc = bacc.Bacc("TRN2", target_bir_lowering=False, debug=False,
                   num_devices=NCORES)

    item = nc.dram_tensor("item", [BT, INUM], F32, kind="ExternalInput")
    inter = nc.dram_tensor("interaction", [BT, IN_DIM], F32, kind="ExternalInput")
    A_w = nc.dram_tensor("A_w", [EDIM, INUM], F32, kind="ExternalInput")
    B_w = nc.dram_tensor("B_w", [EDIM, IN_DIM], F32, kind="ExternalInput")
    kmat = nc.dram_tensor("kmat", [CNUM, EDIM], F32, kind="ExternalInput")
    vmat0 = nc.dram_tensor("vmat0", [CNUM, EDIM], F32, kind="ExternalInput")
    lin_w = nc.dram_tensor("lin_w", [EDIM, 2 * EDIM], F32, kind="ExternalInput")
    lin_b = nc.dram_tensor("lin_b", [EDIM], F32, kind="ExternalInput")
    cls_w = nc.dram_tensor("cls_w", [INUM, EDIM], F32, kind="ExternalInput")
    cls_b = nc.dram_tensor("cls_b", [INUM], F32, kind="ExternalInput")
    er_w = nc.dram_tensor("er_w", [EDIM, EDIM], F32, kind="ExternalInput")
    er_b = nc.dram_tensor("er_b", [EDIM], F32, kind="ExternalInput")
    ad_w = nc.dram_tensor("ad_w", [EDIM, EDIM], F32, kind="ExternalInput")
    ad_b = nc.dram_tensor("ad_b", [EDIM], F32, kind="ExternalInput")
    out = nc.dram_tensor("out", [BT, INUM], F32, kind="ExternalOutput")
    # softmax weights, permuted to [t, b, c] so the scan can DMA one row per step
    w_rt = nc.dram_tensor("w_rt", [S, BL, CNUM], F32, kind="Internal")

    with tile.TileContext(nc) as tc:
        with tc.tile_pool(name="singles", bufs=1) as sg:
            ident = sg.tile([128, 128], F32, tag="ident")
            make_identity(nc, ident[:])
            ones128 = sg.tile([1, 128], F32, tag="ones128")
            nc.vector.memset(ones128[:], 1.0)
            onesK = sg.tile([128, 1], F32, tag="onesK")
            nc.vector.memset(onesK[:], 1.0)
            ones1x50 = sg.tile([1, 50], F32, tag="ones1x50")
            nc.vector.memset(ones1x50[:], 1.0)

            # ---- persistent weights (transposed via PE) ----
            A_wT = sg.tile([128, IK, 128], mybir.dt.float32r, tag="A_wT")
            B_wT = sg.tile([128, DK, 128], mybir.dt.float32r, tag="B_wT")
            kmatT = sg.tile([128, CNUM], F32, tag="kmatT")
            lin_w1T = sg.tile([128, 128], F32, tag="lin_w1T")
            lin_w2T = sg.tile([128, 128], F32, tag="lin_w2T")
            er_wT = sg.tile([128, 128], F32, tag="er_wT")
            ad_wT = sg.tile([128, 128], F32, tag="ad_wT")
            cls_wT = sg.tile([128, INUM], F32, tag="cls_wT")
            V0T = sg.tile([128, CNUM], F32, tag="V0T")
            lin_b_col = sg.tile([128, 1], F32, tag="lin_b_col")
            er_b_col = sg.tile([128, 1], F32, tag="er_b_col")
            ad_b_col = sg.tile([128, 1], F32, tag="ad_b_col")
            cls_b_row = sg.tile([1, INUM], F32, tag="cls_b_row")


            nc.sync.dma_start(lin_b_col[:], lin_b.ap()[:, None])
            nc.sync.dma_start(er_b_col[:], er_b.ap()[:, None])
            nc.sync.dma_start(ad_b_col[:], ad_b.ap()[:, None])
            nc.sync.dma_start(cls_b_row[:], cls_b.ap()[None, :])

            # persistent per-(b,t) activations
            itmT = sg.tile([128, BT], F32, tag="itmT")          # [e, (b,t)]
            eT = sg.tile([128, BL, S], F32, tag="eT")           # [e, b, t]
            aT = sg.tile([128, BL, S], F32, tag="aT")
            rT = sg.tile([128, BL, S], F32, tag="rT")
            E_sb = sg.tile([128, BT], F32, tag="E_sb")          # exp(logits), rows >=50 zero
            wN = sg.tile([CNUM, BT], F32, tag="wN")             # normalized softmax [c,(b,t)]
            Z_sb = sg.tile([1, BT], F32, tag="Z_sb")
            Zr = sg.tile([1, BT], F32, tag="Zr")
            V = sg.tile([128, BL, CNUM], F32, tag="V")
            cls_wTr = sg.tile([128, INUM], mybir.dt.float32r, tag="cls_wTr")
            ones128r = sg.tile([1, 128], mybir.dt.float32r, tag="ones128r")
            cls_b_rowr = sg.tile([1, INUM], mybir.dt.float32r, tag="cls_b_rowr")


            # ---- phase 0: weight transposes ----
            with tc.tile_pool(name="p0sb", bufs=2) as p0, \
                 tc.tile_pool(name="p0ps", bufs=2, space="PSUM") as p0p:
                aw = p0.tile([128, INUM], F32, tag="wld")
                nc.sync.dma_start(aw[:], A_w.ap())
                for k in range(IK):
                    ps = p0p.tile([128, 128], F32, tag="tp")
                    nc.tensor.transpose(ps[:KC, :], aw[:, k * KC:(k + 1) * KC], ident[:])
                    nc.scalar.copy(A_wT[:KC, k, :], ps[:KC, :])
                bw = p0.tile([128, IN_DIM], F32, tag="wld2")
                nc.sync.dma_start(bw[:], B_w.ap())
                for k in range(DK):
                    ps = p0p.tile([128, 128], F32, tag="tp")
                    nc.tensor.transpose(ps[:KC, :], bw[:, k * KC:(k + 1) * KC], ident[:])
                    nc.scalar.copy(B_wT[:KC, k, :], ps[:KC, :])

                km = p0.tile([CNUM, 128], F32, tag="wsm")
                nc.sync.dma_start(km[:], kmat.ap())
                ps = p0p.tile([128, 128], F32, tag="tp")
                nc.tensor.transpose(ps[:, :CNUM], km[:], ident[:CNUM, :CNUM])
                nc.scalar.copy(kmatT[:], ps[:, :CNUM])

                vm = p0.tile([CNUM, 128], F32, tag="wsm")
                nc.sync.dma_start(vm[:], vmat0.ap())
                ps = p0p.tile([128, 128], F32, tag="tp")
                nc.tensor.transpose(ps[:, :CNUM], vm[:], ident[:CNUM, :CNUM])
                nc.scalar.copy(V0T[:], ps[:, :CNUM])

                lw = p0.tile([128, 256], F32, tag="wsm")
                nc.sync.dma_start(lw[:], lin_w.ap())
                ps = p0p.tile([128, 128], F32, tag="tp")
                nc.tensor.transpose(ps[:], lw[:, 0:128], ident[:])
                nc.scalar.copy(lin_w1T[:], ps[:])
                ps = p0p.tile([128, 128], F32, tag="tp")
                nc.tensor.transpose(ps[:], lw[:, 128:256], ident[:])
                nc.scalar.copy(lin_w2T[:], ps[:])

                ew = p0.tile([128, 128], F32, tag="wsm")
                nc.sync.dma_start(ew[:], er_w.ap())
                ps = p0p.tile([128, 128], F32, tag="tp")
                nc.tensor.transpose(ps[:], ew[:], ident[:])
                nc.scalar.copy(er_wT[:], ps[:])

                adw = p0.tile([128, 128], F32, tag="wsm")
                nc.sync.dma_start(adw[:], ad_w.ap())
                ps = p0p.tile([128, 128], F32, tag="tp")
                nc.tensor.transpose(ps[:], adw[:], ident[:])
                nc.scalar.copy(ad_wT[:], ps[:])

                for k in range(IK):
                    cw = p0.tile([KC, 128], F32, tag="wsm")
                    nc.sync.dma_start(cw[:], cls_w.ap()[k * KC:(k + 1) * KC, :])
                    ps = p0p.tile([128, 128], F32, tag="tp")
                    nc.tensor.transpose(ps[:, :KC], cw[:], ident[:KC, :KC])
                    nc.scalar.copy(cls_wTr[:, k * KC:(k + 1) * KC], ps[:, :KC])
                nc.scalar.copy(ones128r[:], ones128[:])
                nc.scalar.copy(cls_b_rowr[:], cls_b_row[:])

            # ---- phase 1: projections over bt-chunks (chunk j == batch j) ----
            with tc.tile_pool(name="p1sb", bufs=2) as p1, \
                 tc.tile_pool(name="p1ps", bufs=2, space="PSUM") as p1p:
                for jp in range(BL // 2):
                    # process a PAIR of batches so the f32r matmuls get a
                    # 400-wide moving operand (full-rate fp32r needs >=256)
                    bt0 = jp * 2 * S
                    cols2 = slice(bt0, bt0 + 2 * S)
                    itemT = p1.tile([128, IK, 2 * S], mybir.dt.float32r, tag="itemT")
                    interT = p1.tile([128, DK, 2 * S], mybir.dt.float32r, tag="interT")
                    for (s0, sw) in ((0, 128), (128, 72), (200, 128), (328, 72)):
                        nat = p1.tile([128, INUM], F32, tag="it_nat")
                        nc.sync.dma_start(nat[:sw], item.ap()[bt0 + s0: bt0 + s0 + sw])
                        for k in range(IK):
                            ps = p1p.tile([128, 128], F32, tag="psT")
                            nc.tensor.transpose(
                                ps[:KC, :sw], nat[:sw, k * KC:(k + 1) * KC],
                                ident[:sw, :sw])
                            if k % 2 == 0:
                                nc.scalar.copy(itemT[:KC, k, s0:s0 + sw], ps[:KC, :sw])
                            else:
                                nc.vector.tensor_copy(itemT[:KC, k, s0:s0 + sw], ps[:KC, :sw])
                        nat2 = p1.tile([128, IN_DIM], F32, tag="in_nat")
                        nc.sync.dma_start(nat2[:sw], inter.ap()[bt0 + s0: bt0 + s0 + sw])
                        for k in range(DK):
                            ps = p1p.tile([128, 128], F32, tag="psT")
                            nc.tensor.transpose(
                                ps[:KC, :sw], nat2[:sw, k * KC:(k + 1) * KC],
                                ident[:sw, :sw])
                            if k % 2 == 0:
                                nc.scalar.copy(interT[:KC, k, s0:s0 + sw], ps[:KC, :sw])
                            else:
                                nc.vector.tensor_copy(interT[:KC, k, s0:s0 + sw], ps[:KC, :sw])

                    ps_itm = p1p.tile([128, 2 * S], F32, tag="ps_itm")
                    for k in range(IK):
                        nc.tensor.matmul(ps_itm[:], A_wT[:KC, k, :], itemT[:KC, k, :],
                                         start=(k == 0), stop=(k == IK - 1))
                    nc.scalar.copy(itmT[:, cols2], ps_itm[:])

                    ps_itr = p1p.tile([128, 2 * S], F32, tag="ps_itr")
                    for k in range(DK):
                        nc.tensor.matmul(ps_itr[:], B_wT[:KC, k, :], interT[:KC, k, :],
                                         start=(k == 0), stop=(k == DK - 1))
                    itr_t = p1.tile([128, 2 * S], F32, tag="itr_t")
                    nc.scalar.copy(itr_t[:], ps_itr[:])

                    j2 = jp * 2
                    ps_e = p1p.tile([128, 2 * S], F32, tag="ps_eal")
                    nc.tensor.matmul(ps_e[:], er_wT[:], itr_t[:], start=True, stop=True)
                    nc.scalar.activation(eT[:, j2:j2 + 2, :], ps_e[:], AF.Sigmoid,
                                         bias=er_b_col[:], scale=1.0)
                    ps_a = p1p.tile([128, 2 * S], F32, tag="ps_eal")
                    nc.tensor.matmul(ps_a[:], ad_wT[:], itr_t[:], start=True, stop=True)
                    nc.scalar.activation(aT[:, j2:j2 + 2, :], ps_a[:], AF.Tanh,
                                         bias=ad_b_col[:], scale=1.0)
                    ps_l = p1p.tile([128, 2 * S], F32, tag="ps_eal")
                    nc.tensor.matmul(ps_l[:CNUM, :], kmatT[:], itmT[:, cols2],
                                     start=True, stop=True)
                    nc.scalar.activation(E_sb[:CNUM, cols2], ps_l[:CNUM, :], AF.Exp)

            # ---- softmax normalization + w layout roundtrip ----
            with tc.tile_pool(name="smsb", bufs=2) as sm, \
                 tc.tile_pool(name="smps", bufs=2, space="PSUM") as smp:
                for q in range(4):
                    qc = slice(q * 400, q * 400 + 400)
                    ps_z = smp.tile([1, 400], F32, tag="ps_z")
                    nc.tensor.matmul(ps_z[:], onesK[:CNUM], E_sb[:CNUM, qc],
                                     start=True, stop=True)
                    nc.scalar.copy(Z_sb[:, qc], ps_z[:])
                nc.vector.reciprocal(Zr[:], Z_sb[:])
                for q in range(4):
                    qc = slice(q * 400, q * 400 + 400)
                    ps_zb = smp.tile([CNUM, 400], F32, tag="ps_zb")
                    nc.tensor.matmul(ps_zb[:], ones1x50[:], Zr[:, qc], start=True, stop=True)
                    nc.vector.scalar_tensor_tensor(
                        out=wN[:, qc], in0=E_sb[:CNUM, qc], scalar=1.0, in1=ps_zb[:],
                        op0=OP.mult, op1=OP.mult)
                # wN [c,(b,t)] -> DRAM [t, b, c] via PE transposes + strided store
                for b in range(BL):
                    for (t0, tw) in ((0, 128), (128, 72)):
                        u0 = b * S + t0
                        ps_w = smp.tile([128, CNUM], F32, tag="ps_wt")
                        nc.tensor.transpose(ps_w[:tw, :], wN[:, u0:u0 + tw],
                                            ident[:CNUM, :CNUM])
                        wbt = sm.tile([128, CNUM], F32, tag="wbt")
                        nc.scalar.copy(wbt[:tw], ps_w[:tw, :])
                        nc.sync.dma_start(w_rt.ap()[t0:t0 + tw, b, :], wbt[:tw])

            # ---- scan init ----
            for b in range(BL):
                nc.scalar.copy(V[:, b, :], V0T[:])

            # ---- phase 2: the scan ----
            with tc.tile_pool(name="scsb", bufs=3) as sc, \
                 tc.tile_pool(name="scps", bufs=2, space="PSUM") as scp:
                for t in range(S):
                    w_row = sc.tile([1, BL, CNUM], F32, tag="w_row")
                    nc.sync.dma_start(w_row[:], w_rt.ap()[t:t + 1])
                    ps_w = scp.tile([128, BL, CNUM], F32, tag="psw")
                    nc.tensor.matmul(ps_w[:], ones128[:], w_row[:],
                                     start=True, stop=True)
                    X = sc.tile([128, BL, CNUM], F32, tag="X")
                    nc.vector.scalar_tensor_tensor(
                        out=X[:], in0=V[:], scalar=1.0, in1=ps_w[:],
                        op0=OP.mult, op1=OP.mult)
                    nc.vector.tensor_reduce(
                        out=rT[:, :, t], in_=X[:], axis=mybir.AxisListType.X,
                        op=OP.add)
                    e_bv = eT[:, :, t:t + 1].to_broadcast([128, BL, CNUM])
                    a_bv = aT[:, :, t:t + 1].to_broadcast([128, BL, CNUM])
                    P = sc.tile([128, BL, CNUM], F32, tag="P")
                    nc.vector.scalar_tensor_tensor(
                        out=P[:], in0=X[:], scalar=1.0, in1=e_bv,
                        op0=OP.mult, op1=OP.mult)
                    Y = sc.tile([128, BL, CNUM], F32, tag="Y")
                    nc.vector.scalar_tensor_tensor(
                        out=Y[:], in0=P[:], scalar=-1.0, in1=V[:],
                        op0=OP.mult, op1=OP.add)
                    Q = sc.tile([128, BL, CNUM], F32, tag="Q")
                    nc.vector.scalar_tensor_tensor(
                        out=Q[:], in0=ps_w[:], scalar=1.0, in1=a_bv,
                        op0=OP.mult, op1=OP.mult)
                    nc.vector.scalar_tensor_tensor(
                        out=V[:], in0=Q[:], scalar=1.0, in1=Y[:],
                        op0=OP.mult, op1=OP.add)

            # ---- phase 3: h + output ----
            with tc.tile_pool(name="p3sb", bufs=2) as p3, \
                 tc.tile_pool(name="p3ps", bufs=2, space="PSUM") as p3p:
                for j in range(BL):
                    bt0 = j * S
                    cols = slice(bt0, bt0 + S)
                    ps_h = p3p.tile([128, S], F32, tag="ps_h")
                    nc.tensor.matmul(ps_h[:], lin_w1T[:], rT[:, j, :],
                                     start=True, stop=False)
                    nc.tensor.matmul(ps_h[:], lin_w2T[:], itmT[:, cols],
                                     start=False, stop=True)
                    hT = p3.tile([128, S], mybir.dt.float32r, tag="hT")
                    nc.scalar.activation(hT[:], ps_h[:], AF.Tanh,
                                         bias=lin_b_col[:], scale=1.0)
                    for (s0, sw) in ((0, 128), (128, 72)):
                        ot = p3.tile([128, INUM], F32, tag="ot")
                        for half in range(2):
                            hc = slice(half * 500, half * 500 + 500)
                            ps_o = p3p.tile([128, 500], F32, tag="ps_o")
                            nc.tensor.matmul(ps_o[:sw, :], hT[:, s0:s0 + sw],
                                             cls_wTr[:, hc], start=True, stop=False)
                            nc.tensor.matmul(ps_o[:sw, :], ones128r[:, :sw],
                                             cls_b_rowr[:, hc], start=False, stop=True)
                            nc.scalar.activation(ot[:sw, hc], ps_o[:sw, :], AF.Sigmoid)
                        nc.sync.dma_start(out.ap()[bt0 + s0: bt0 + s0 + sw], ot[:sw])

    nc.compile()
    return nc


def kernel(**inputs):
    global LAST_RESULT
    if "nc" not in _NC_CACHE:
        _NC_CACHE["nc"] = _build()
    nc = _NC_CACHE["nc"]

    shared = {k: np.ascontiguousarray(np.asarray(inputs[k], dtype=np.float32))
              for k in ("A_w", "B_w", "kmat", "vmat0", "lin_w", "lin_b",
                        "cls_w", "cls_b", "er_w", "er_b", "ad_w", "ad_b")}
    item = np.asarray(inputs["item"], dtype=np.float32)
    inter = np.asarray(inputs["interaction"], dtype=np.float32)

    in_maps = []
    for c in range(NCORES):
        m = dict(shared)
        m["item"] = np.ascontiguousarray(
            item[c * BL:(c + 1) * BL].reshape(BT, INUM))
        m["interaction"] = np.ascontiguousarray(
            inter[c * BL:(c + 1) * BL].reshape(BT, IN_DIM))
        in_maps.append(m)

    res = run_bass_kernel_spmd(nc, in_maps, core_ids=list(range(NCORES)))
    LAST_RESULT = res
    outs = [res.results[c]["out"].reshape(BL, S, INUM) for c in range(NCORES)]
    return np.concatenate(outs, axis=0)



# revision 6
# speedup vs baseline: 1.6993x; 1.6993x over previous
"""DKVMN kernel for Trainium2 (8 NeuronCores, data-parallel over batch).

Shapes (hardcoded): B=64, S=200, INUM=1000, IN_DIM=2000, CNUM=50, EDIM=128.

Per core: B_loc = 8 batches, BT = 1600 flat (b,t) steps. Host pre-transposes
and bf16-casts item/interaction and all weights, so no on-device transposes.

Phase 1 (per b-pair, bt-chunks of 400):
    itmT [e,bt] = A_wT.T @ itemT   (bf16 matmuls, contraction chunks of 125)
    itrT [e,bt] = B_wT.T @ interT
    e16 = sigmoid(er_wT.T @ itrT + er_b), a16 = tanh(ad_wT.T @ itrT + ad_b)
    logits = kmatT.T @ itm16; E16 = exp(logits)
    softmax: Z via ones-matmul, reciprocal, w16 = E16 * Zbc
Phase 2 (per b, bulk over the (c,t)=10000 free dim; t innermost):
    Wbc16[e,c,t] = w[c,t] broadcast via ones-matmul (PE) + ACT copy
    FM = Wbc*e_bv (DVE TT 2x, fp16) ; M = 1-FM (DVE TSP 4x, in-place)
    A16 = Wbc*a_bv (DVE/Pool split)
    t=0 column fixup: A[:, :, 0] += M0*V0 ; M[:, :, 0] = 0
    V16 = tensor_tensor_scan(M, A)  — state_t = M_t*state + A_t (fp32 state)
    X16 = Wbc*V_{t-1} (DVE TT 2x, shifted view)
    r-projection: hps += sum_c lin1T @ X16[:,c,:] (50 accumulating PE matmuls)
Phase 3 (per b-pair): hps += lin2T @ itm16; h=tanh(+lin_b);
    out = sigmoid(h.T @ cls_wT + cls_b) via PE + ACT, DMA out.
"""

import numpy as np
import ml_dtypes

import concourse.bass as bass
import concourse.mybir as mybir
import concourse.tile as tile
from concourse import bacc
from concourse.bass_utils import run_bass_kernel_spmd

F32 = mybir.dt.float32
BF16 = mybir.dt.bfloat16
FP16 = mybir.dt.float16
AF = mybir.ActivationFunctionType
OP = mybir.AluOpType

B, S, INUM, IN_DIM, CNUM, EDIM = 64, 200, 1000, 2000, 50, 128
NCORES = 8
BL = B // NCORES          # 8 batches per core
BT = BL * S               # 1600
IK = 8                    # INUM k-chunks of 125
DK = 16                   # IN_DIM k-chunks of 125
KC = 125
CT = CNUM * S             # 10000
WQ = 20                   # Wbc chunks of 500
WC = CT // WQ             # 500

# per-b engine assignments ("dve" or "pool") for the bulk elementwise
# passes; the scan itself must run on DVE (gpsimd scan fails NEFF codegen)
FM_ENG = ["dve", "pool", "pool", "dve", "pool", "pool", "dve", "pool"]
A_ENG = ["pool"] * 8
X_ENG = ["dve", "pool", "pool", "dve", "pool", "pool", "dve", "pool"]

_NC_CACHE = {}
LAST_RESULT = None


def _eng(nc, name):
    return nc.vector if name == "dve" else nc.gpsimd


def _build():
    nc = bacc.Bacc("TRN2", target_bir_lowering=False, debug=False,
                   num_devices=NCORES)

    itemT = nc.dram_tensor("itemT", [INUM, BT], BF16, kind="ExternalInput")
    interT = nc.dram_tensor("interT", [IN_DIM, BT], BF16, kind="ExternalInput")
    A_wT = nc.dram_tensor("A_wT", [INUM, EDIM], BF16, kind="ExternalInput")
    B_wT = nc.dram_tensor("B_wT", [IN_DIM, EDIM], BF16, kind="ExternalInput")
    kmatT = nc.dram_tensor("kmatT", [EDIM, CNUM], BF16, kind="ExternalInput")
    er_wT = nc.dram_tensor("er_wT", [EDIM, EDIM], BF16, kind="ExternalInput")
    ad_wT = nc.dram_tensor("ad_wT", [EDIM, EDIM], BF16, kind="ExternalInput")
    lin1T = nc.dram_tensor("lin1T", [EDIM, EDIM], BF16, kind="ExternalInput")
    lin2T = nc.dram_tensor("lin2T", [EDIM, EDIM], BF16, kind="ExternalInput")
    cls_wT = nc.dram_tensor("cls_wT", [EDIM, INUM], BF16, kind="ExternalInput")
    v0T16 = nc.dram_tensor("v0T16", [EDIM, CNUM], BF16, kind="ExternalInput")
    lin_b = nc.dram_tensor("lin_b", [EDIM], F32, kind="ExternalInput")
    er_b = nc.dram_tensor("er_b", [EDIM], F32, kind="ExternalInput")
    ad_b = nc.dram_tensor("ad_b", [EDIM], F32, kind="ExternalInput")
    cls_b16 = nc.dram_tensor("cls_b16", [1, INUM], BF16, kind="ExternalInput")
    out = nc.dram_tensor("out", [BT, INUM], F32, kind="ExternalOutput")
    w_r = nc.dram_tensor("w_r", [BL, CNUM, S], BF16, kind="Internal")

    with tile.TileContext(nc) as tc:
        with tc.tile_pool(name="singles", bufs=1) as sg:
            ones16 = sg.tile([1, 128], BF16, tag="ones16")
            nc.vector.memset(ones16[:], 1.0)
            ones50 = sg.tile([CNUM, 1], BF16, tag="ones50")
            nc.vector.memset(ones50[:], 1.0)

            A_w_sb = sg.tile([KC, IK, EDIM], BF16, tag="A_w_sb")
            B_w_sb = sg.tile([KC, DK, EDIM], BF16, tag="B_w_sb")
            kmat_sb = sg.tile([EDIM, CNUM], BF16, tag="kmat_sb")
            er_w_sb = sg.tile([EDIM, EDIM], BF16, tag="er_w_sb")
            ad_w_sb = sg.tile([EDIM, EDIM], BF16, tag="ad_w_sb")
            lin1_sb = sg.tile([EDIM, EDIM], BF16, tag="lin1_sb")
            lin2_sb = sg.tile([EDIM, EDIM], BF16, tag="lin2_sb")
            cls_w_sb = sg.tile([EDIM, INUM], BF16, tag="cls_w_sb")
            v0_sb = sg.tile([EDIM, CNUM], BF16, tag="v0_sb")
            lin_b_col = sg.tile([EDIM, 1], F32, tag="lin_b_col")
            er_b_col = sg.tile([EDIM, 1], F32, tag="er_b_col")
            ad_b_col = sg.tile([EDIM, 1], F32, tag="ad_b_col")
            cls_b_sb = sg.tile([1, INUM], BF16, tag="cls_b_sb")

            for k in range(IK):
                nc.sync.dma_start(A_w_sb[:, k, :],
                                  A_wT.ap()[k * KC:(k + 1) * KC, :])
            for k in range(DK):
                nc.sync.dma_start(B_w_sb[:, k, :],
                                  B_wT.ap()[k * KC:(k + 1) * KC, :])
            nc.sync.dma_start(kmat_sb[:], kmatT.ap())
            nc.sync.dma_start(er_w_sb[:], er_wT.ap())
            nc.sync.dma_start(ad_w_sb[:], ad_wT.ap())
            nc.sync.dma_start(lin1_sb[:], lin1T.ap())
            nc.sync.dma_start(lin2_sb[:], lin2T.ap())
            nc.sync.dma_start(cls_w_sb[:], cls_wT.ap())
            nc.sync.dma_start(v0_sb[:], v0T16.ap())
            nc.sync.dma_start(lin_b_col[:], lin_b.ap()[:, None])
            nc.sync.dma_start(er_b_col[:], er_b.ap()[:, None])
            nc.sync.dma_start(ad_b_col[:], ad_b.ap()[:, None])
            nc.sync.dma_start(cls_b_sb[:], cls_b16.ap())

            # persistent activations
            itm16 = sg.tile([EDIM, BT], BF16, tag="itm16")
            e16 = sg.tile([EDIM, BL, S], BF16, tag="e16")
            a16 = sg.tile([EDIM, BL, S], BF16, tag="a16")
            E16 = sg.tile([CNUM, BT], BF16, tag="E16")
            w16 = sg.tile([CNUM, BT], BF16, tag="w16")
            Z_sb = sg.tile([1, BT], F32, tag="Z_sb")
            Zr16 = sg.tile([1, BT], BF16, tag="Zr16")

            # ---- phase 1: projections over bt-chunks of 400 ----
            with tc.tile_pool(name="p1sb", bufs=2) as p1, \
                 tc.tile_pool(name="p1ps", bufs=2, space="PSUM") as p1p:
                for jp in range(BL // 2):
                    c0 = jp * 2 * S
                    cols = slice(c0, c0 + 2 * S)
                    it_j = p1.tile([KC, IK, 2 * S], BF16, tag="it_j")
                    for k in range(IK):
                        nc.sync.dma_start(it_j[:, k, :],
                                          itemT.ap()[k * KC:(k + 1) * KC, cols])
                    in_j = p1.tile([KC, DK, 2 * S], BF16, tag="in_j")
                    for k in range(DK):
                        nc.sync.dma_start(in_j[:, k, :],
                                          interT.ap()[k * KC:(k + 1) * KC, cols])

                    ps_itm = p1p.tile([EDIM, 2 * S], F32, tag="ps_itm")
                    for k in range(IK):
                        nc.tensor.matmul(ps_itm[:], A_w_sb[:, k, :], it_j[:, k, :],
                                         start=(k == 0), stop=(k == IK - 1))
                    nc.scalar.copy(itm16[:, cols], ps_itm[:])

                    ps_itr = p1p.tile([EDIM, 2 * S], F32, tag="ps_itr")
                    for k in range(DK):
                        nc.tensor.matmul(ps_itr[:], B_w_sb[:, k, :], in_j[:, k, :],
                                         start=(k == 0), stop=(k == DK - 1))
                    itr_j = p1.tile([EDIM, 2 * S], BF16, tag="itr_j")
                    nc.scalar.copy(itr_j[:], ps_itr[:])

                    j2 = jp * 2
                    ps_e = p1p.tile([EDIM, 2 * S], F32, tag="ps_ea")
                    nc.tensor.matmul(ps_e[:], er_w_sb[:], itr_j[:],
                                     start=True, stop=True)
                    nc.scalar.activation(e16[:, j2:j2 + 2, :], ps_e[:],
                                         AF.Sigmoid, bias=er_b_col[:], scale=1.0)
                    ps_a = p1p.tile([EDIM, 2 * S], F32, tag="ps_ea")
                    nc.tensor.matmul(ps_a[:], ad_w_sb[:], itr_j[:],
                                     start=True, stop=True)
                    nc.scalar.activation(a16[:, j2:j2 + 2, :], ps_a[:],
                                         AF.Tanh, bias=ad_b_col[:], scale=1.0)
                    ps_l = p1p.tile([CNUM, 2 * S], F32, tag="ps_l")
                    nc.tensor.matmul(ps_l[:], kmat_sb[:], itm16[:, cols],
                                     start=True, stop=True)
                    nc.scalar.activation(E16[:, cols], ps_l[:], AF.Exp)

            # ---- softmax normalization ----
            with tc.tile_pool(name="smsb", bufs=2) as sm, \
                 tc.tile_pool(name="smps", bufs=2, space="PSUM") as smp:
                for q in range(4):
                    qc = slice(q * 400, q * 400 + 400)
                    ps_z = smp.tile([1, 400], F32, tag="ps_z")
                    nc.tensor.matmul(ps_z[:], ones50[:], E16[:, qc],
                                     start=True, stop=True)
                    nc.scalar.copy(Z_sb[:, qc], ps_z[:])
                Zr = sm.tile([1, BT], F32, tag="Zr")
                nc.vector.reciprocal(Zr[:], Z_sb[:])
                nc.vector.tensor_copy(Zr16[:], Zr[:])
                for q in range(4):
                    qc = slice(q * 400, q * 400 + 400)
                    ps_zb = smp.tile([CNUM, 400], F32, tag="ps_zb")
                    nc.tensor.matmul(ps_zb[:], ones16[:1, :CNUM], Zr16[:, qc],
                                     start=True, stop=True)
                    nc.vector.scalar_tensor_tensor(
                        out=w16[:, qc], in0=E16[:, qc], scalar=1.0,
                        in1=ps_zb[:], op0=OP.mult, op1=OP.mult)
                # w16 [c,(b,t)] -> DRAM [b, c, t] (per-b flat rows for the
                # ones-broadcast moving operand)
                for b in range(BL):
                    nc.sync.dma_start(w_r.ap()[b],
                                      w16[:, b * S:(b + 1) * S])

            # ---- phase 2: bulk scan per b ----
            with tc.tile_pool(name="scW", bufs=2) as scW, \
                 tc.tile_pool(name="scFM", bufs=2) as scFM, \
                 tc.tile_pool(name="scA", bufs=2) as scA, \
                 tc.tile_pool(name="scV", bufs=1) as scV, \
                 tc.tile_pool(name="scX", bufs=1) as scX, \
                 tc.tile_pool(name="scwr", bufs=4) as scwr, \
                 tc.tile_pool(name="scps", bufs=3, space="PSUM") as scp, \
                 tc.tile_pool(name="hps_pool", bufs=2, space="PSUM") as hpsp, \
                 tc.tile_pool(name="clsps", bufs=2, space="PSUM") as clsp, \
                 tc.tile_pool(name="p3sb", bufs=2) as p3:
                hps = None
                for b in range(BL):
                    # W broadcast: [1,500] rows -> PSUM [128,500] -> SBUF bf16
                    Wbc = scW.tile([EDIM, CNUM, S], BF16, tag="Wbc")
                    w_flat = w_r.ap()[b:b + 1].rearrange("a c t -> a (c t)")
                    for q in range(WQ):
                        wrow = scwr.tile([1, WC], BF16, tag="wrow")
                        nc.sync.dma_start(wrow[:], w_flat[:, q * WC:(q + 1) * WC])
                        ps_w = scp.tile([EDIM, WC], F32, tag="ps_w")
                        nc.tensor.matmul(ps_w[:], ones16[:], wrow[:],
                                         start=True, stop=True)
                        nc.scalar.copy(
                            Wbc[:].rearrange("p c t -> p (c t)")[
                                :, q * WC:(q + 1) * WC], ps_w[:])

                    e_bv = e16[:, b:b + 1, :].to_broadcast([EDIM, CNUM, S])
                    a_bv = a16[:, b:b + 1, :].to_broadcast([EDIM, CNUM, S])

                    FM = scFM.tile([EDIM, CNUM, S], FP16, tag="FM")
                    _eng(nc, FM_ENG[b]).tensor_tensor(out=FM[:], in0=Wbc[:],
                                                      in1=e_bv, op=OP.mult)
                    nc.vector.tensor_scalar(out=FM[:], in0=FM[:],
                                            scalar1=-1.0, scalar2=1.0,
                                            op0=OP.mult, op1=OP.add)
                    A = scA.tile([EDIM, CNUM, S], BF16, tag="A")
                    _eng(nc, A_ENG[b]).tensor_tensor(out=A[:], in0=Wbc[:],
                                                     in1=a_bv, op=OP.mult)
                    # t=0 fixup: A0 += M0*V0 ; M0 = 0
                    t0 = scwr.tile([EDIM, CNUM, 1], F32, tag="t0")
                    nc.vector.scalar_tensor_tensor(
                        out=t0[:], in0=FM[:, :, 0:1], scalar=1.0,
                        in1=v0_sb[:, :, None], op0=OP.mult, op1=OP.mult)
                    nc.vector.tensor_tensor(out=A[:, :, 0:1], in0=A[:, :, 0:1],
                                            in1=t0[:], op=OP.add)
                    nc.vector.tensor_scalar(out=FM[:, :, 0:1], in0=FM[:, :, 0:1],
                                            scalar1=0.0, scalar2=None,
                                            op0=OP.mult)

                    V = scV.tile([EDIM, CNUM, S], BF16, tag="V")
                    nc.vector.tensor_tensor_scan(
                        out=V[:].rearrange("p c t -> p (c t)"),
                        data0=FM[:].rearrange("p c t -> p (c t)"),
                        data1=A[:].rearrange("p c t -> p (c t)"),
                        initial=0.0, op0=OP.mult, op1=OP.add)

                    X = scX.tile([EDIM, CNUM, S], BF16, tag="X")
                    _eng(nc, X_ENG[b]).tensor_tensor(
                        out=X[:, :, 1:], in0=Wbc[:, :, 1:],
                        in1=V[:, :, 0:S - 1], op=OP.mult)
                    nc.vector.tensor_tensor(out=X[:, :, 0:1], in0=Wbc[:, :, 0:1],
                                            in1=v0_sb[:, :, None], op=OP.mult)

                    # r-projection: hps[:, half] += sum_c lin1 @ X[:,c,:]
                    half = b % 2
                    if half == 0:
                        hps = hpsp.tile([EDIM, 2 * S], F32, tag="hps")
                    hcols = slice(half * S, half * S + S)
                    for c in range(CNUM):
                        nc.tensor.matmul(hps[:, hcols], lin1_sb[:], X[:, c, :],
                                         start=(c == 0), stop=False)
                    nc.tensor.matmul(hps[:, hcols], lin2_sb[:],
                                     itm16[:, b * S:(b + 1) * S],
                                     start=False, stop=True)

                    # ---- phase 3 per completed pair ----
                    if half == 1:
                        c0 = (b - 1) * S
                        h16 = p3.tile([EDIM, 2 * S], BF16, tag="h16")
                        nc.scalar.activation(h16[:], hps[:], AF.Tanh,
                                             bias=lin_b_col[:], scale=1.0)
                        for (s0, sw) in ((0, 128), (128, 72),
                                         (200, 128), (328, 72)):
                            ot = p3.tile([128, INUM], F32, tag="ot")
                            for hf in range(2):
                                hc = slice(hf * 500, hf * 500 + 500)
                                ps_o = clsp.tile([128, 500], F32, tag="ps_o")
                                nc.tensor.matmul(ps_o[:sw], h16[:, s0:s0 + sw],
                                                 cls_w_sb[:, hc],
                                                 start=True, stop=False)
                                nc.tensor.matmul(ps_o[:sw], ones16[:1, :sw],
                                                 cls_b_sb[:, hc],
                                                 start=False, stop=True)
                                nc.scalar.activation(ot[:sw, hc], ps_o[:sw],
                                                     AF.Sigmoid)
                            nc.sync.dma_start(
                                out.ap()[c0 + s0: c0 + s0 + sw], ot[:sw])

    nc.compile()
    return nc


def kernel(**inputs):
    global LAST_RESULT
    if "nc" not in _NC_CACHE:
        _NC_CACHE["nc"] = _build()
    nc = _NC_CACHE["nc"]

    bf = ml_dtypes.bfloat16
    f32 = {k: np.asarray(inputs[k], dtype=np.float32) for k in inputs}
    lin_w = f32["lin_w"]
    shared = {
        "A_wT": np.ascontiguousarray(f32["A_w"].T).astype(bf),
        "B_wT": np.ascontiguousarray(f32["B_w"].T).astype(bf),
        "kmatT": np.ascontiguousarray(f32["kmat"].T).astype(bf),
        "er_wT": np.ascontiguousarray(f32["er_w"].T).astype(bf),
        "ad_wT": np.ascontiguousarray(f32["ad_w"].T).astype(bf),
        "lin1T": np.ascontiguousarray(lin_w[:, :EDIM].T).astype(bf),
        "lin2T": np.ascontiguousarray(lin_w[:, EDIM:].T).astype(bf),
        "cls_wT": np.ascontiguousarray(f32["cls_w"].T).astype(bf),
        "v0T16": np.ascontiguousarray(f32["vmat0"].T).astype(bf),
        "lin_b": f32["lin_b"],
        "er_b": f32["er_b"],
        "ad_b": f32["ad_b"],
        "cls_b16": np.ascontiguousarray(f32["cls_b"][None, :]).astype(bf),
    }
    item = f32["item"]
    inter = f32["interaction"]

    in_maps = []
    for c in range(NCORES):
        m = dict(shared)
        m["itemT"] = np.ascontiguousarray(
            item[c * BL:(c + 1) * BL].reshape(BT, INUM).T).astype(bf)
        m["interT"] = np.ascontiguousarray(
            inter[c * BL:(c + 1) * BL].reshape(BT, IN_DIM).T).astype(bf)
        in_maps.append(m)

    res = run_bass_kernel_spmd(nc, in_maps, core_ids=list(range(NCORES)))
    LAST_RESULT = res
    outs = [res.results[c]["out"].reshape(BL, S, INUM) for c in range(NCORES)]
    return np.concatenate(outs, axis=0)


# revision 11
# speedup vs baseline: 2.3434x; 1.3790x over previous
"""DKVMN kernel for Trainium2 (8 NeuronCores, data-parallel over batch).

Shapes (hardcoded): B=64, S=200, INUM=1000, IN_DIM=2000, CNUM=50, EDIM=128.

Per core: B_loc = 8 batches, BT = 1600 flat (b,t) steps. Host pre-transposes
and bf16-casts item/interaction and all weights, so no on-device transposes.

Phase 1 (per b-pair, bt-chunks of 400):
    itmT [e,bt] = A_wT.T @ itemT   (bf16 matmuls, contraction chunks of 125)
    itrT [e,bt] = B_wT.T @ interT
    e16 = sigmoid(er_wT.T @ itrT + er_b), a16 = tanh(ad_wT.T @ itrT + ad_b)
    logits = kmatT.T @ itm16; E16 = exp(logits)
    softmax: Z via ones-matmul, reciprocal, w16 = E16 * Zbc
Phase 2 (per b, bulk over the (c,t)=10000 free dim; t innermost):
    Wbc16[e,c,t] = w[c,t] broadcast via ones-matmul (PE) + ACT copy
    FM = Wbc*e_bv (DVE TT 2x, fp16) ; M = 1-FM (DVE TSP 4x, in-place)
    A16 = Wbc*a_bv (DVE/Pool split)
    t=0 column fixup: A[:, :, 0] += M0*V0 ; M[:, :, 0] = 0
    V16 = tensor_tensor_scan(M, A)  — state_t = M_t*state + A_t (fp32 state)
    X16 = Wbc*V_{t-1} (DVE TT 2x, shifted view)
    r-projection: hps += sum_c lin1T @ X16[:,c,:] (50 accumulating PE matmuls)
Phase 3 (per b-pair): hps += lin2T @ itm16; h=tanh(+lin_b);
    out = sigmoid(h.T @ cls_wT + cls_b) via PE + ACT, DMA out.
"""

import numpy as np
import ml_dtypes

import concourse.bass as bass
import concourse.mybir as mybir
import concourse.tile as tile
from concourse import bacc
from concourse.bass_utils import run_bass_kernel_spmd

F32 = mybir.dt.float32
BF16 = mybir.dt.bfloat16
FP16 = mybir.dt.float16
AF = mybir.ActivationFunctionType
OP = mybir.AluOpType

B, S, INUM, IN_DIM, CNUM, EDIM = 64, 200, 1000, 2000, 50, 128
NCORES = 8
BL = B // NCORES          # 8 batches per core
BT = BL * S               # 1600
IK = 8                    # INUM k-chunks of 125
DK = 16                   # IN_DIM k-chunks of 125
KC = 125
CT = CNUM * S             # 10000
WQ = 20                   # Wbc chunks of 500
WC = CT // WQ             # 500

# per-b engine assignments ("dve" or "pool") for the bulk elementwise
# passes; the scan itself must run on DVE (gpsimd scan fails NEFF codegen)
# and gpsimd multiplies run at 0.42 efficiency (~19.8us/pass), so Pool only
# absorbs a few A passes.
FM_ENG = ["dve"] * 8
A_ENG = ["dve", "pool", "pool", "pool", "dve", "pool", "pool", "pool"]
X_ENG = ["dve"] * 8

_NC_CACHE = {}
LAST_RESULT = None


def _eng(nc, name):
    return nc.vector if name == "dve" else nc.gpsimd


def _build():
    nc = bacc.Bacc("TRN2", target_bir_lowering=False, debug=False,
                   num_devices=NCORES)

    itemT = nc.dram_tensor("itemT", [INUM, BT], BF16, kind="ExternalInput")
    interT = nc.dram_tensor("interT", [IN_DIM, BT], BF16, kind="ExternalInput")
    A_wT = nc.dram_tensor("A_wT", [INUM, EDIM], BF16, kind="ExternalInput")
    B_wT = nc.dram_tensor("B_wT", [IN_DIM, EDIM], BF16, kind="ExternalInput")
    kmatT = nc.dram_tensor("kmatT", [EDIM, CNUM], BF16, kind="ExternalInput")
    er_wT = nc.dram_tensor("er_wT", [EDIM, EDIM], BF16, kind="ExternalInput")
    ad_wT = nc.dram_tensor("ad_wT", [EDIM, EDIM], BF16, kind="ExternalInput")
    lin1T = nc.dram_tensor("lin1T", [EDIM, EDIM], BF16, kind="ExternalInput")
    lin2T = nc.dram_tensor("lin2T", [EDIM, EDIM], BF16, kind="ExternalInput")
    cls_wT = nc.dram_tensor("cls_wT", [EDIM, INUM], BF16, kind="ExternalInput")
    v0T16 = nc.dram_tensor("v0T16", [EDIM, CNUM], BF16, kind="ExternalInput")
    lin_b = nc.dram_tensor("lin_b", [EDIM], F32, kind="ExternalInput")
    er_b = nc.dram_tensor("er_b", [EDIM], F32, kind="ExternalInput")
    ad_b = nc.dram_tensor("ad_b", [EDIM], F32, kind="ExternalInput")
    cls_b16 = nc.dram_tensor("cls_b16", [1, INUM], BF16, kind="ExternalInput")
    out = nc.dram_tensor("out", [BT, INUM], F32, kind="ExternalOutput")
    w_r = nc.dram_tensor("w_r", [BL, CNUM, S], BF16, kind="Internal")

    with tile.TileContext(nc) as tc:
        with tc.tile_pool(name="singles", bufs=1) as sg:
            ones16 = sg.tile([1, 128], BF16, tag="ones16")
            nc.vector.memset(ones16[:], 1.0)
            ones50 = sg.tile([CNUM, 1], BF16, tag="ones50")
            nc.vector.memset(ones50[:], 1.0)

            A_w_sb = sg.tile([KC, IK, EDIM], BF16, tag="A_w_sb")
            B_w_sb = sg.tile([KC, DK, EDIM], BF16, tag="B_w_sb")
            kmat_sb = sg.tile([EDIM, CNUM], BF16, tag="kmat_sb")
            er_w_sb = sg.tile([EDIM, EDIM], BF16, tag="er_w_sb")
            ad_w_sb = sg.tile([EDIM, EDIM], BF16, tag="ad_w_sb")
            lin1_sb = sg.tile([EDIM, EDIM], BF16, tag="lin1_sb")
            lin2_sb = sg.tile([EDIM, EDIM], BF16, tag="lin2_sb")
            cls_w_sb = sg.tile([EDIM, INUM], BF16, tag="cls_w_sb")
            v0_sb = sg.tile([EDIM, CNUM], BF16, tag="v0_sb")
            lin_b_col = sg.tile([EDIM, 1], F32, tag="lin_b_col")
            er_b_col = sg.tile([EDIM, 1], F32, tag="er_b_col")
            ad_b_col = sg.tile([EDIM, 1], F32, tag="ad_b_col")
            cls_b_sb = sg.tile([1, INUM], BF16, tag="cls_b_sb")

            for k in range(IK):
                nc.sync.dma_start(A_w_sb[:, k, :],
                                  A_wT.ap()[k * KC:(k + 1) * KC, :])
            for k in range(DK):
                nc.sync.dma_start(B_w_sb[:, k, :],
                                  B_wT.ap()[k * KC:(k + 1) * KC, :])
            nc.sync.dma_start(kmat_sb[:], kmatT.ap())
            nc.sync.dma_start(er_w_sb[:], er_wT.ap())
            nc.sync.dma_start(ad_w_sb[:], ad_wT.ap())
            nc.sync.dma_start(lin1_sb[:], lin1T.ap())
            nc.sync.dma_start(lin2_sb[:], lin2T.ap())
            nc.sync.dma_start(cls_w_sb[:], cls_wT.ap())
            nc.sync.dma_start(v0_sb[:], v0T16.ap())
            nc.sync.dma_start(lin_b_col[:], lin_b.ap()[:, None])
            nc.sync.dma_start(er_b_col[:], er_b.ap()[:, None])
            nc.sync.dma_start(ad_b_col[:], ad_b.ap()[:, None])
            nc.sync.dma_start(cls_b_sb[:], cls_b16.ap())

            # persistent activations
            itm16 = sg.tile([EDIM, BT], BF16, tag="itm16")
            e16 = sg.tile([EDIM, BL, S], BF16, tag="e16")
            a16 = sg.tile([EDIM, BL, S], BF16, tag="a16")
            E16 = sg.tile([CNUM, BT], BF16, tag="E16")
            w16 = sg.tile([CNUM, BT], BF16, tag="w16")
            Z_sb = sg.tile([1, BT], F32, tag="Z_sb")
            Zr16 = sg.tile([1, BT], BF16, tag="Zr16")

            # ---- phase 1: projections over bt-chunks of 400 ----
            with tc.tile_pool(name="p1sb", bufs=2) as p1, \
                 tc.tile_pool(name="p1ps", bufs=2, space="PSUM") as p1p:
                for jp in range(BL // 2):
                    c0 = jp * 2 * S
                    cols = slice(c0, c0 + 2 * S)
                    it_j = p1.tile([KC, IK, 2 * S], BF16, tag="it_j")
                    for k in range(IK):
                        nc.sync.dma_start(it_j[:, k, :],
                                          itemT.ap()[k * KC:(k + 1) * KC, cols])
                    in_j = p1.tile([KC, DK, 2 * S], BF16, tag="in_j")
                    for k in range(DK):
                        nc.sync.dma_start(in_j[:, k, :],
                                          interT.ap()[k * KC:(k + 1) * KC, cols])

                    ps_itm = p1p.tile([EDIM, 2 * S], F32, tag="ps_itm")
                    for k in range(IK):
                        nc.tensor.matmul(ps_itm[:], A_w_sb[:, k, :], it_j[:, k, :],
                                         start=(k == 0), stop=(k == IK - 1))
                    nc.scalar.copy(itm16[:, cols], ps_itm[:])

                    ps_itr = p1p.tile([EDIM, 2 * S], F32, tag="ps_itr")
                    for k in range(DK):
                        nc.tensor.matmul(ps_itr[:], B_w_sb[:, k, :], in_j[:, k, :],
                                         start=(k == 0), stop=(k == DK - 1))
                    itr_j = p1.tile([EDIM, 2 * S], BF16, tag="itr_j")
                    nc.scalar.copy(itr_j[:], ps_itr[:])

                    j2 = jp * 2
                    ps_e = p1p.tile([EDIM, 2 * S], F32, tag="ps_ea")
                    nc.tensor.matmul(ps_e[:], er_w_sb[:], itr_j[:],
                                     start=True, stop=True)
                    nc.scalar.activation(e16[:, j2:j2 + 2, :], ps_e[:],
                                         AF.Sigmoid, bias=er_b_col[:], scale=1.0)
                    ps_a = p1p.tile([EDIM, 2 * S], F32, tag="ps_ea")
                    nc.tensor.matmul(ps_a[:], ad_w_sb[:], itr_j[:],
                                     start=True, stop=True)
                    nc.scalar.activation(a16[:, j2:j2 + 2, :], ps_a[:],
                                         AF.Tanh, bias=ad_b_col[:], scale=1.0)
                    ps_l = p1p.tile([CNUM, 2 * S], F32, tag="ps_l")
                    nc.tensor.matmul(ps_l[:], kmat_sb[:], itm16[:, cols],
                                     start=True, stop=True)
                    nc.scalar.activation(E16[:, cols], ps_l[:], AF.Exp)

            # ---- softmax normalization ----
            with tc.tile_pool(name="smsb", bufs=2) as sm, \
                 tc.tile_pool(name="smps", bufs=2, space="PSUM") as smp:
                for q in range(4):
                    qc = slice(q * 400, q * 400 + 400)
                    ps_z = smp.tile([1, 400], F32, tag="ps_z")
                    nc.tensor.matmul(ps_z[:], ones50[:], E16[:, qc],
                                     start=True, stop=True)
                    nc.scalar.copy(Z_sb[:, qc], ps_z[:])
                Zr = sm.tile([1, BT], F32, tag="Zr")
                nc.vector.reciprocal(Zr[:], Z_sb[:])
                nc.vector.tensor_copy(Zr16[:], Zr[:])
                for q in range(4):
                    qc = slice(q * 400, q * 400 + 400)
                    ps_zb = smp.tile([CNUM, 400], F32, tag="ps_zb")
                    nc.tensor.matmul(ps_zb[:], ones16[:1, :CNUM], Zr16[:, qc],
                                     start=True, stop=True)
                    nc.vector.scalar_tensor_tensor(
                        out=w16[:, qc], in0=E16[:, qc], scalar=1.0,
                        in1=ps_zb[:], op0=OP.mult, op1=OP.mult)
                # w16 [c,(b,t)] -> DRAM [b, c, t] (per-b flat rows for the
                # ones-broadcast moving operand)
                for b in range(BL):
                    nc.sync.dma_start(w_r.ap()[b],
                                      w16[:, b * S:(b + 1) * S])

            # ---- phase 2: bulk scan per b ----
            with tc.tile_pool(name="scW", bufs=2) as scW, \
                 tc.tile_pool(name="scFM", bufs=1) as scFM, \
                 tc.tile_pool(name="scA", bufs=1) as scA, \
                 tc.tile_pool(name="scV", bufs=1) as scV, \
                 tc.tile_pool(name="scX", bufs=1) as scX, \
                 tc.tile_pool(name="scwr", bufs=2) as scwr, \
                 tc.tile_pool(name="scps", bufs=2, space="PSUM") as scp, \
                 tc.tile_pool(name="hps_pool", bufs=2, space="PSUM") as hpsp, \
                 tc.tile_pool(name="clsps", bufs=2, space="PSUM") as clsp, \
                 tc.tile_pool(name="p3sb", bufs=2) as p3:
                hps = None
                XP = None
                for b in range(BL):
                    # W broadcast: [1,2500] rows -> PSUM [128,1000] via two
                    # 500-col ones-matmuls -> one SBUF bf16 copy per 1000
                    Wbc = scW.tile([EDIM, CNUM, S], BF16, tag="Wbc")
                    Wbc_f = Wbc[:].rearrange("p c t -> p (c t)")
                    w_flat = w_r.ap()[b:b + 1].rearrange("a c t -> a (c t)")
                    for ql in range(5):
                        wrow = scwr.tile([1, 2000], BF16, tag="wrow")
                        nc.sync.dma_start(
                            wrow[:], w_flat[:, ql * 2000:(ql + 1) * 2000])
                        for qh in range(2):
                            # [2, 512] keeps each 500-col matmul bank-aligned
                            ps_w = scp.tile([EDIM, 2, 512], F32, tag="ps_w")
                            q0 = qh * 1000
                            for qq in range(2):
                                nc.tensor.matmul(
                                    ps_w[:, qq, 0:500],
                                    ones16[:],
                                    wrow[:, q0 + qq * 500:q0 + (qq + 1) * 500],
                                    start=True, stop=True)
                            dst = Wbc_f[
                                :, ql * 2000 + q0:ql * 2000 + q0 + 1000
                            ].rearrange("p (a b) -> p a b", a=2)
                            nc.scalar.copy(dst, ps_w[:, :, 0:500])

                    e_bv = e16[:, b:b + 1, :].to_broadcast([EDIM, CNUM, S])
                    a_bv = a16[:, b:b + 1, :].to_broadcast([EDIM, CNUM, S])

                    FM = scFM.tile([EDIM, CNUM, S], FP16, tag="FM")
                    _eng(nc, FM_ENG[b]).tensor_tensor(out=FM[:], in0=Wbc[:],
                                                      in1=e_bv, op=OP.mult)
                    nc.vector.tensor_scalar(out=FM[:], in0=FM[:],
                                            scalar1=-1.0, scalar2=1.0,
                                            op0=OP.mult, op1=OP.add)
                    A = scA.tile([EDIM, CNUM, S], BF16, tag="A")
                    _eng(nc, A_ENG[b]).tensor_tensor(out=A[:], in0=Wbc[:],
                                                     in1=a_bv, op=OP.mult)
                    # t=0 fixup: A0 += M0*V0 ; M0 = 0
                    t0 = scwr.tile([EDIM, CNUM, 1], F32, tag="t0")
                    nc.vector.scalar_tensor_tensor(
                        out=t0[:], in0=FM[:, :, 0:1], scalar=1.0,
                        in1=v0_sb[:, :, None], op0=OP.mult, op1=OP.mult)
                    nc.vector.tensor_tensor(out=A[:, :, 0:1], in0=A[:, :, 0:1],
                                            in1=t0[:], op=OP.add)
                    nc.vector.tensor_scalar(out=FM[:, :, 0:1], in0=FM[:, :, 0:1],
                                            scalar1=0.0, scalar2=None,
                                            op0=OP.mult)

                    V = scV.tile([EDIM, CNUM, S], BF16, tag="V")
                    nc.vector.tensor_tensor_scan(
                        out=V[:].rearrange("p c t -> p (c t)"),
                        data0=FM[:].rearrange("p c t -> p (c t)"),
                        data1=A[:].rearrange("p c t -> p (c t)"),
                        initial=0.0, op0=OP.mult, op1=OP.add)

                    half = b % 2
                    if half == 0:
                        XP = scX.tile([EDIM, CNUM, 2, S], BF16, tag="XP")
                    _eng(nc, X_ENG[b]).tensor_tensor(
                        out=XP[:, :, half, 1:], in0=Wbc[:, :, 1:],
                        in1=V[:, :, 0:S - 1], op=OP.mult)
                    nc.vector.tensor_tensor(out=XP[:, :, half, 0:1],
                                            in0=Wbc[:, :, 0:1],
                                            in1=v0_sb[:, :, None], op=OP.mult)

                    # ---- phase 3 per completed pair ----
                    if half == 1:
                        # r-projection for the pair:
                        # hps[:, (half,t)] = sum_c lin1 @ XP[:,c,:,:] + lin2 @ itm
                        hps = hpsp.tile([EDIM, 2 * S], F32, tag="hps")
                        for c in range(CNUM):
                            nc.tensor.matmul(hps[:], lin1_sb[:],
                                             XP[:, c, :, :],
                                             start=(c == 0), stop=False)
                        nc.tensor.matmul(hps[:], lin2_sb[:],
                                         itm16[:, (b - 1) * S:(b + 1) * S],
                                         start=False, stop=True)
                        c0 = (b - 1) * S
                        h16 = p3.tile([EDIM, 2 * S], BF16, tag="h16")
                        nc.scalar.activation(h16[:], hps[:], AF.Tanh,
                                             bias=lin_b_col[:], scale=1.0)
                        for (s0, sw) in ((0, 128), (128, 72),
                                         (200, 128), (328, 72)):
                            ot = p3.tile([128, INUM], F32, tag="ot")
                            for hf in range(2):
                                hc = slice(hf * 500, hf * 500 + 500)
                                ps_o = clsp.tile([128, 500], F32, tag="ps_o")
                                nc.tensor.matmul(ps_o[:sw], h16[:, s0:s0 + sw],
                                                 cls_w_sb[:, hc],
                                                 start=True, stop=False)
                                nc.tensor.matmul(ps_o[:sw], ones16[:1, :sw],
                                                 cls_b_sb[:, hc],
                                                 start=False, stop=True)
                                nc.scalar.activation(ot[:sw, hc], ps_o[:sw],
                                                     AF.Sigmoid)
                            nc.sync.dma_start(
                                out.ap()[c0 + s0: c0 + s0 + sw], ot[:sw])

    nc.compile()
    return nc


def kernel(**inputs):
    global LAST_RESULT
    if "nc" not in _NC_CACHE:
        _NC_CACHE["nc"] = _build()
    nc = _NC_CACHE["nc"]

    bf = ml_dtypes.bfloat16
    f32 = {k: np.asarray(inputs[k], dtype=np.float32) for k in inputs}
    lin_w = f32["lin_w"]
    shared = {
        "A_wT": np.ascontiguousarray(f32["A_w"].T).astype(bf),
        "B_wT": np.ascontiguousarray(f32["B_w"].T).astype(bf),
        "kmatT": np.ascontiguousarray(f32["kmat"].T).astype(bf),
        "er_wT": np.ascontiguousarray(f32["er_w"].T).astype(bf),
        "ad_wT": np.ascontiguousarray(f32["ad_w"].T).astype(bf),
        "lin1T": np.ascontiguousarray(lin_w[:, :EDIM].T).astype(bf),
        "lin2T": np.ascontiguousarray(lin_w[:, EDIM:].T).astype(bf),
        "cls_wT": np.ascontiguousarray(f32["cls_w"].T).astype(bf),
        "v0T16": np.ascontiguousarray(f32["vmat0"].T).astype(bf),
        "lin_b": f32["lin_b"],
        "er_b": f32["er_b"],
        "ad_b": f32["ad_b"],
        "cls_b16": np.ascontiguousarray(f32["cls_b"][None, :]).astype(bf),
    }
    item = f32["item"]
    inter = f32["interaction"]

    in_maps = []
    for c in range(NCORES):
        m = dict(shared)
        m["itemT"] = np.ascontiguousarray(
            item[c * BL:(c + 1) * BL].reshape(BT, INUM).T).astype(bf)
        m["interT"] = np.ascontiguousarray(
            inter[c * BL:(c + 1) * BL].reshape(BT, IN_DIM).T).astype(bf)
        in_maps.append(m)

    res = run_bass_kernel_spmd(nc, in_maps, core_ids=list(range(NCORES)))
    LAST_RESULT = res
    outs = [res.results[c]["out"].reshape(BL, S, INUM) for c in range(NCORES)]
    return np.concatenate(outs, axis=0)


# revision 15
# speedup vs baseline: 2.7463x; 1.1719x over previous
"""DKVMN kernel for Trainium2 (8 NeuronCores, data-parallel over batch).

Shapes (hardcoded): B=64, S=200, INUM=1000, IN_DIM=2000, CNUM=50, EDIM=128.

Per core: B_loc = 8 batches, BT = 1600 flat (b,t) steps. Host pre-transposes
and bf16-casts item/interaction and all weights, so no on-device transposes.

Phase 1 (per b-pair, bt-chunks of 400):
    itmT [e,bt] = A_wT.T @ itemT   (bf16 matmuls, contraction chunks of 125)
    itrT [e,bt] = B_wT.T @ interT
    e16 = sigmoid(er_wT.T @ itrT + er_b), a16 = tanh(ad_wT.T @ itrT + ad_b)
    logits = kmatT.T @ itm16; E16 = exp(logits)
    softmax: Z via ones-matmul, reciprocal, w16 = E16 * Zbc
Phase 2 (per b, bulk over the (c,t)=10000 free dim; t innermost):
    Wbc16[e,c,t] = w[c,t] broadcast via ones-matmul (PE) + ACT copy
    FM = Wbc*e_bv (DVE TT 2x, fp16) ; M = 1-FM (DVE TSP 4x, in-place)
    A16 = Wbc*a_bv (DVE/Pool split)
    t=0 column fixup: A[:, :, 0] += M0*V0 ; M[:, :, 0] = 0
    V16 = tensor_tensor_scan(M, A)  — state_t = M_t*state + A_t (fp32 state)
    X16 = Wbc*V_{t-1} (DVE TT 2x, shifted view)
    r-projection: hps += sum_c lin1T @ X16[:,c,:] (50 accumulating PE matmuls)
Phase 3 (per b-pair): hps += lin2T @ itm16; h=tanh(+lin_b);
    out = sigmoid(h.T @ cls_wT + cls_b) via PE + ACT, DMA out.
"""

import numpy as np
import ml_dtypes

import concourse.bass as bass
import concourse.mybir as mybir
import concourse.tile as tile
from concourse import bacc
from concourse.bass_utils import run_bass_kernel_spmd

F32 = mybir.dt.float32
BF16 = mybir.dt.bfloat16
FP16 = mybir.dt.float16
AF = mybir.ActivationFunctionType
OP = mybir.AluOpType

B, S, INUM, IN_DIM, CNUM, EDIM = 64, 200, 1000, 2000, 50, 128
NCORES = 8
BL = B // NCORES          # 8 batches per core
BT = BL * S               # 1600
IK = 8                    # INUM k-chunks of 125
DK = 16                   # IN_DIM k-chunks of 125
KC = 125
CT = CNUM * S             # 10000
WQ = 20                   # Wbc chunks of 500
WC = CT // WQ             # 500

# per-b engine assignments ("dve" or "pool") for the bulk elementwise
# passes; the scan itself must run on DVE (gpsimd scan fails NEFF codegen)
# and gpsimd multiplies run at 0.42 efficiency (~19.8us/pass), so Pool only
# absorbs a few A passes.
FM_ENG = ["dve"] * 8
A_ENG = ["dve", "pool", "pool", "pool", "dve", "pool", "pool", "pool"]
X_ENG = ["dve"] * 8

_NC_CACHE = {}
LAST_RESULT = None


def _eng(nc, name):
    return nc.vector if name == "dve" else nc.gpsimd


def _build():
    nc = bacc.Bacc("TRN2", target_bir_lowering=False, debug=False,
                   num_devices=NCORES)

    itemT = nc.dram_tensor("itemT", [INUM, BT], BF16, kind="ExternalInput")
    interT = nc.dram_tensor("interT", [IN_DIM, BT], BF16, kind="ExternalInput")
    A_wT = nc.dram_tensor("A_wT", [INUM, EDIM], BF16, kind="ExternalInput")
    B_wT = nc.dram_tensor("B_wT", [IN_DIM, EDIM], BF16, kind="ExternalInput")
    kmatT = nc.dram_tensor("kmatT", [EDIM, CNUM], BF16, kind="ExternalInput")
    er_wT = nc.dram_tensor("er_wT", [EDIM, EDIM], BF16, kind="ExternalInput")
    ad_wT = nc.dram_tensor("ad_wT", [EDIM, EDIM], BF16, kind="ExternalInput")
    lin1T = nc.dram_tensor("lin1T", [EDIM, EDIM], BF16, kind="ExternalInput")
    lin2T = nc.dram_tensor("lin2T", [EDIM, EDIM], BF16, kind="ExternalInput")
    cls_wT = nc.dram_tensor("cls_wT", [EDIM, INUM], BF16, kind="ExternalInput")
    v0T16 = nc.dram_tensor("v0T16", [EDIM, CNUM], BF16, kind="ExternalInput")
    lin_b = nc.dram_tensor("lin_b", [EDIM], F32, kind="ExternalInput")
    er_b = nc.dram_tensor("er_b", [EDIM], F32, kind="ExternalInput")
    ad_b = nc.dram_tensor("ad_b", [EDIM], F32, kind="ExternalInput")
    cls_b16 = nc.dram_tensor("cls_b16", [1, INUM], BF16, kind="ExternalInput")
    out = nc.dram_tensor("out", [BT, INUM], F32, kind="ExternalOutput")
    w_r = nc.dram_tensor("w_r", [BL, CNUM, S], BF16, kind="Internal")

    with tile.TileContext(nc) as tc:
        with tc.tile_pool(name="singles", bufs=1) as sg:
            ones16 = sg.tile([1, 128], BF16, tag="ones16")
            nc.vector.memset(ones16[:], 1.0)
            ones50 = sg.tile([CNUM, 1], BF16, tag="ones50")
            nc.vector.memset(ones50[:], 1.0)

            A_w_sb = sg.tile([KC, IK, EDIM], BF16, tag="A_w_sb")
            B_w_sb = sg.tile([KC, DK, EDIM], BF16, tag="B_w_sb")
            kmat_sb = sg.tile([EDIM, CNUM], BF16, tag="kmat_sb")
            er_w_sb = sg.tile([EDIM, EDIM], BF16, tag="er_w_sb")
            ad_w_sb = sg.tile([EDIM, EDIM], BF16, tag="ad_w_sb")
            lin1_sb = sg.tile([EDIM, EDIM], BF16, tag="lin1_sb")
            lin2_sb = sg.tile([EDIM, EDIM], BF16, tag="lin2_sb")
            cls_w_sb = sg.tile([EDIM, INUM], BF16, tag="cls_w_sb")
            v0_sb = sg.tile([EDIM, CNUM], BF16, tag="v0_sb")
            lin_b_col = sg.tile([EDIM, 1], F32, tag="lin_b_col")
            er_b_col = sg.tile([EDIM, 1], F32, tag="er_b_col")
            ad_b_col = sg.tile([EDIM, 1], F32, tag="ad_b_col")
            cls_b_sb = sg.tile([1, INUM], BF16, tag="cls_b_sb")

            for k in range(IK):
                nc.sync.dma_start(A_w_sb[:, k, :],
                                  A_wT.ap()[k * KC:(k + 1) * KC, :])
            for k in range(DK):
                nc.sync.dma_start(B_w_sb[:, k, :],
                                  B_wT.ap()[k * KC:(k + 1) * KC, :])
            nc.sync.dma_start(kmat_sb[:], kmatT.ap())
            nc.sync.dma_start(er_w_sb[:], er_wT.ap())
            nc.sync.dma_start(ad_w_sb[:], ad_wT.ap())
            nc.sync.dma_start(lin1_sb[:], lin1T.ap())
            nc.sync.dma_start(lin2_sb[:], lin2T.ap())
            nc.sync.dma_start(cls_w_sb[:], cls_wT.ap())
            nc.sync.dma_start(v0_sb[:], v0T16.ap())
            nc.sync.dma_start(lin_b_col[:], lin_b.ap()[:, None])
            nc.sync.dma_start(er_b_col[:], er_b.ap()[:, None])
            nc.sync.dma_start(ad_b_col[:], ad_b.ap()[:, None])
            nc.sync.dma_start(cls_b_sb[:], cls_b16.ap())

            # persistent activations
            itm16 = sg.tile([EDIM, BT], BF16, tag="itm16")
            e16 = sg.tile([EDIM, BL, S], BF16, tag="e16")
            a16 = sg.tile([EDIM, BL, S], BF16, tag="a16")
            E16 = sg.tile([CNUM, BT], BF16, tag="E16")
            w16 = sg.tile([CNUM, BT], BF16, tag="w16")
            Z_sb = sg.tile([1, BT], F32, tag="Z_sb")
            Zr16 = sg.tile([1, BT], BF16, tag="Zr16")

            # ---- phase 1: projections over bt-chunks of 400 ----
            with tc.tile_pool(name="p1sb", bufs=2) as p1, \
                 tc.tile_pool(name="p1ps", bufs=2, space="PSUM") as p1p:
                for jp in range(BL // 2):
                    c0 = jp * 2 * S
                    cols = slice(c0, c0 + 2 * S)
                    it_j = p1.tile([KC, IK, 2 * S], BF16, tag="it_j")
                    for k in range(IK):
                        nc.sync.dma_start(it_j[:, k, :],
                                          itemT.ap()[k * KC:(k + 1) * KC, cols])
                    in_j = p1.tile([KC, DK, 2 * S], BF16, tag="in_j")
                    for k in range(DK):
                        nc.sync.dma_start(in_j[:, k, :],
                                          interT.ap()[k * KC:(k + 1) * KC, cols])

                    ps_itm = p1p.tile([EDIM, 2 * S], F32, tag="ps_itm")
                    for k in range(IK):
                        nc.tensor.matmul(ps_itm[:], A_w_sb[:, k, :], it_j[:, k, :],
                                         start=(k == 0), stop=(k == IK - 1))
                    nc.scalar.copy(itm16[:, cols], ps_itm[:])

                    ps_itr = p1p.tile([EDIM, 2 * S], F32, tag="ps_itr")
                    for k in range(DK):
                        nc.tensor.matmul(ps_itr[:], B_w_sb[:, k, :], in_j[:, k, :],
                                         start=(k == 0), stop=(k == DK - 1))
                    itr_j = p1.tile([EDIM, 2 * S], BF16, tag="itr_j")
                    nc.scalar.copy(itr_j[:], ps_itr[:])

                    j2 = jp * 2
                    ps_e = p1p.tile([EDIM, 2 * S], F32, tag="ps_ea")
                    nc.tensor.matmul(ps_e[:], er_w_sb[:], itr_j[:],
                                     start=True, stop=True)
                    nc.scalar.activation(e16[:, j2:j2 + 2, :], ps_e[:],
                                         AF.Sigmoid, bias=er_b_col[:], scale=1.0)
                    ps_a = p1p.tile([EDIM, 2 * S], F32, tag="ps_ea")
                    nc.tensor.matmul(ps_a[:], ad_w_sb[:], itr_j[:],
                                     start=True, stop=True)
                    nc.scalar.activation(a16[:, j2:j2 + 2, :], ps_a[:],
                                         AF.Tanh, bias=ad_b_col[:], scale=1.0)
                    ps_l = p1p.tile([CNUM, 2 * S], F32, tag="ps_l")
                    nc.tensor.matmul(ps_l[:], kmat_sb[:], itm16[:, cols],
                                     start=True, stop=True)
                    nc.scalar.activation(E16[:, cols], ps_l[:], AF.Exp)

            # ---- softmax normalization ----
            with tc.tile_pool(name="smsb", bufs=2) as sm, \
                 tc.tile_pool(name="smps", bufs=2, space="PSUM") as smp:
                for q in range(4):
                    qc = slice(q * 400, q * 400 + 400)
                    ps_z = smp.tile([1, 400], F32, tag="ps_z")
                    nc.tensor.matmul(ps_z[:], ones50[:], E16[:, qc],
                                     start=True, stop=True)
                    nc.scalar.copy(Z_sb[:, qc], ps_z[:])
                Zr = sm.tile([1, BT], F32, tag="Zr")
                nc.vector.reciprocal(Zr[:], Z_sb[:])
                nc.vector.tensor_copy(Zr16[:], Zr[:])
                for q in range(4):
                    qc = slice(q * 400, q * 400 + 400)
                    ps_zb = smp.tile([CNUM, 400], F32, tag="ps_zb")
                    nc.tensor.matmul(ps_zb[:], ones16[:1, :CNUM], Zr16[:, qc],
                                     start=True, stop=True)
                    nc.vector.scalar_tensor_tensor(
                        out=w16[:, qc], in0=E16[:, qc], scalar=1.0,
                        in1=ps_zb[:], op0=OP.mult, op1=OP.mult)
                # w16 [c,(b,t)] -> DRAM [b, c, t] (per-b flat rows for the
                # ones-broadcast moving operand)
                for b in range(BL):
                    nc.sync.dma_start(w_r.ap()[b],
                                      w16[:, b * S:(b + 1) * S])

            # ---- phase 2: bulk scan per b ----
            with tc.tile_pool(name="scW", bufs=2) as scW, \
                 tc.tile_pool(name="scFM", bufs=1) as scFM, \
                 tc.tile_pool(name="scA", bufs=2) as scA, \
                 tc.tile_pool(name="scV", bufs=1) as scV, \
                 tc.tile_pool(name="scX", bufs=1) as scX, \
                 tc.tile_pool(name="hps_pool", bufs=2, space="PSUM") as hpsp, \
                 tc.tile_pool(name="clsps", bufs=2, space="PSUM") as clsp, \
                 tc.tile_pool(name="p3sb", bufs=2) as p3:
                hps = None
                XP = None
                for b in range(BL):
                    # W broadcast across partitions: one stride-0 DMA per b
                    Wbc = scW.tile([EDIM, CNUM, S], BF16, tag="Wbc")
                    w_flat = w_r.ap()[b:b + 1].rearrange("a c t -> a (c t)")
                    nc.sync.dma_start(
                        Wbc[:].rearrange("p c t -> p (c t)"),
                        w_flat.to_broadcast([EDIM, CT]))

                    e_bv = e16[:, b:b + 1, :].to_broadcast([EDIM, CNUM, S])
                    a_bv = a16[:, b:b + 1, :].to_broadcast([EDIM, CNUM, S])

                    A = scA.tile([EDIM, CNUM, S], BF16, tag="A")
                    _eng(nc, A_ENG[b]).tensor_tensor(out=A[:], in0=Wbc[:],
                                                     in1=a_bv, op=OP.mult)
                    FM = scFM.tile([EDIM, CNUM, S], FP16, tag="FM")
                    _eng(nc, FM_ENG[b]).tensor_tensor(out=FM[:], in0=Wbc[:],
                                                      in1=e_bv, op=OP.mult)
                    nc.vector.tensor_scalar(out=FM[:], in0=FM[:],
                                            scalar1=-1.0, scalar2=1.0,
                                            op0=OP.mult, op1=OP.add)
                    # t=0 fixup: A0 += M0*V0 ; M0 = 0
                    t0 = p3.tile([EDIM, CNUM, 1], F32, tag="t0")
                    nc.vector.scalar_tensor_tensor(
                        out=t0[:], in0=FM[:, :, 0:1], scalar=1.0,
                        in1=v0_sb[:, :, None], op0=OP.mult, op1=OP.mult)
                    nc.vector.tensor_tensor(out=A[:, :, 0:1], in0=A[:, :, 0:1],
                                            in1=t0[:], op=OP.add)
                    nc.vector.tensor_scalar(out=FM[:, :, 0:1], in0=FM[:, :, 0:1],
                                            scalar1=0.0, scalar2=None,
                                            op0=OP.mult)

                    V = scV.tile([EDIM, CNUM, S], BF16, tag="V")
                    nc.vector.tensor_tensor_scan(
                        out=V[:].rearrange("p c t -> p (c t)"),
                        data0=FM[:].rearrange("p c t -> p (c t)"),
                        data1=A[:].rearrange("p c t -> p (c t)"),
                        initial=0.0, op0=OP.mult, op1=OP.add)

                    half = b % 2
                    if half == 0:
                        XP = scX.tile([EDIM, CNUM, 2, S], BF16, tag="XP")
                    _eng(nc, X_ENG[b]).tensor_tensor(
                        out=XP[:, :, half, 1:], in0=Wbc[:, :, 1:],
                        in1=V[:, :, 0:S - 1], op=OP.mult)
                    nc.vector.tensor_tensor(out=XP[:, :, half, 0:1],
                                            in0=Wbc[:, :, 0:1],
                                            in1=v0_sb[:, :, None], op=OP.mult)

                    # ---- phase 3 per completed pair ----
                    if half == 1:
                        # r-projection for the pair:
                        # hps[:, (half,t)] = sum_c lin1 @ XP[:,c,:,:] + lin2 @ itm
                        hps = hpsp.tile([EDIM, 2 * S], F32, tag="hps")
                        for c in range(CNUM):
                            nc.tensor.matmul(hps[:], lin1_sb[:],
                                             XP[:, c, :, :],
                                             start=(c == 0), stop=False)
                        nc.tensor.matmul(hps[:], lin2_sb[:],
                                         itm16[:, (b - 1) * S:(b + 1) * S],
                                         start=False, stop=True)
                        c0 = (b - 1) * S
                        h16 = p3.tile([EDIM, 2 * S], BF16, tag="h16")
                        nc.scalar.activation(h16[:], hps[:], AF.Tanh,
                                             bias=lin_b_col[:], scale=1.0)
                        for (s0, sw) in ((0, 128), (128, 72),
                                         (200, 128), (328, 72)):
                            ot = p3.tile([128, INUM], F32, tag="ot")
                            for hf in range(2):
                                hc = slice(hf * 500, hf * 500 + 500)
                                ps_o = clsp.tile([128, 500], F32, tag="ps_o")
                                nc.tensor.matmul(ps_o[:sw], h16[:, s0:s0 + sw],
                                                 cls_w_sb[:, hc],
                                                 start=True, stop=False)
                                nc.tensor.matmul(ps_o[:sw], ones16[:1, :sw],
                                                 cls_b_sb[:, hc],
                                                 start=False, stop=True)
                                nc.scalar.activation(ot[:sw, hc], ps_o[:sw],
                                                     AF.Sigmoid)
                            nc.sync.dma_start(
                                out.ap()[c0 + s0: c0 + s0 + sw], ot[:sw])

    nc.compile()
    return nc


def kernel(**inputs):
    global LAST_RESULT
    if "nc" not in _NC_CACHE:
        _NC_CACHE["nc"] = _build()
    nc = _NC_CACHE["nc"]

    bf = ml_dtypes.bfloat16
    f32 = {k: np.asarray(inputs[k], dtype=np.float32) for k in inputs}
    lin_w = f32["lin_w"]
    shared = {
        "A_wT": np.ascontiguousarray(f32["A_w"].T).astype(bf),
        "B_wT": np.ascontiguousarray(f32["B_w"].T).astype(bf),
        "kmatT": np.ascontiguousarray(f32["kmat"].T).astype(bf),
        "er_wT": np.ascontiguousarray(f32["er_w"].T).astype(bf),
        "ad_wT": np.ascontiguousarray(f32["ad_w"].T).astype(bf),
        "lin1T": np.ascontiguousarray(lin_w[:, :EDIM].T).astype(bf),
        "lin2T": np.ascontiguousarray(lin_w[:, EDIM:].T).astype(bf),
        "cls_wT": np.ascontiguousarray(f32["cls_w"].T).astype(bf),
        "v0T16": np.ascontiguousarray(f32["vmat0"].T).astype(bf),
        "lin_b": f32["lin_b"],
        "er_b": f32["er_b"],
        "ad_b": f32["ad_b"],
        "cls_b16": np.ascontiguousarray(f32["cls_b"][None, :]).astype(bf),
    }
    item = f32["item"]
    inter = f32["interaction"]

    in_maps = []
    for c in range(NCORES):
        m = dict(shared)
        m["itemT"] = np.ascontiguousarray(
            item[c * BL:(c + 1) * BL].reshape(BT, INUM).T).astype(bf)
        m["interT"] = np.ascontiguousarray(
            inter[c * BL:(c + 1) * BL].reshape(BT, IN_DIM).T).astype(bf)
        in_maps.append(m)

    res = run_bass_kernel_spmd(nc, in_maps, core_ids=list(range(NCORES)))
    LAST_RESULT = res
    outs = [res.results[c]["out"].reshape(BL, S, INUM) for c in range(NCORES)]
    return np.concatenate(outs, axis=0)


# revision 21
# speedup vs baseline: 2.9347x; 1.0686x over previous
"""DKVMN kernel for Trainium2 (8 NeuronCores, data-parallel over batch).

Shapes (hardcoded): B=64, S=200, INUM=1000, IN_DIM=2000, CNUM=50, EDIM=128.

Per core: B_loc = 8 batches, BT = 1600 flat (b,t) steps. Host pre-transposes
and bf16-casts item/interaction and all weights, so no on-device transposes.

Phase 1 (per b-pair, bt-chunks of 400):
    itmT [e,bt] = A_wT.T @ itemT   (bf16 matmuls, contraction chunks of 125)
    itrT [e,bt] = B_wT.T @ interT
    e16 = sigmoid(er_wT.T @ itrT + er_b), a16 = tanh(ad_wT.T @ itrT + ad_b)
    logits = kmatT.T @ itm16; E16 = exp(logits)
    softmax: Z via ones-matmul, reciprocal, w16 = E16 * Zbc
Phase 2 (per b, bulk over the (c,t)=10000 free dim; t innermost):
    Wbc16[e,c,t] = w[c,t] broadcast via ones-matmul (PE) + ACT copy
    FM = Wbc*e_bv (DVE TT 2x, fp16) ; M = 1-FM (DVE TSP 4x, in-place)
    A16 = Wbc*a_bv (DVE/Pool split)
    t=0 column fixup: A[:, :, 0] += M0*V0 ; M[:, :, 0] = 0
    V16 = tensor_tensor_scan(M, A)  — state_t = M_t*state + A_t (fp32 state)
    X16 = Wbc*V_{t-1} (DVE TT 2x, shifted view)
    r-projection: hps += sum_c lin1T @ X16[:,c,:] (50 accumulating PE matmuls)
Phase 3 (per b-pair): hps += lin2T @ itm16; h=tanh(+lin_b);
    out = sigmoid(h.T @ cls_wT + cls_b) via PE + ACT, DMA out.
"""

import numpy as np
import ml_dtypes

import concourse.bass as bass
import concourse.mybir as mybir
import concourse.tile as tile
from concourse import bacc
from concourse.bass_utils import run_bass_kernel_spmd

F32 = mybir.dt.float32
BF16 = mybir.dt.bfloat16
FP16 = mybir.dt.float16
AF = mybir.ActivationFunctionType
OP = mybir.AluOpType

B, S, INUM, IN_DIM, CNUM, EDIM = 64, 200, 1000, 2000, 50, 128
NCORES = 8
BL = B // NCORES          # 8 batches per core
BT = BL * S               # 1600
IK = 8                    # INUM k-chunks of 125
DK = 16                   # IN_DIM k-chunks of 125
KC = 125
CT = CNUM * S             # 10000
WQ = 20                   # Wbc chunks of 500
WC = CT // WQ             # 500

# per-b engine assignments ("dve" or "pool") for the bulk elementwise
# passes; the scan itself must run on DVE (gpsimd scan fails NEFF codegen)
# and gpsimd multiplies run at 0.42 efficiency (~19.8us/pass), so Pool only
# absorbs a few A passes.
FM_ENG = ["dve", "dve", "dve", "pool", "dve", "dve", "dve", "dve"]
A_ENG = ["pool"] * 8
X_ENG = ["dve"] * 8

_NC_CACHE = {}
LAST_RESULT = None


def _eng(nc, name):
    return nc.vector if name == "dve" else nc.gpsimd


def _build():
    nc = bacc.Bacc("TRN2", target_bir_lowering=False, debug=False,
                   num_devices=NCORES)

    itemT = nc.dram_tensor("itemT", [INUM, BT], BF16, kind="ExternalInput")
    interT = nc.dram_tensor("interT", [IN_DIM, BT], BF16, kind="ExternalInput")
    A_wT = nc.dram_tensor("A_wT", [INUM, EDIM], BF16, kind="ExternalInput")
    B_wT = nc.dram_tensor("B_wT", [IN_DIM, EDIM], BF16, kind="ExternalInput")
    kmatT = nc.dram_tensor("kmatT", [EDIM, CNUM], BF16, kind="ExternalInput")
    er_wT = nc.dram_tensor("er_wT", [EDIM, EDIM], BF16, kind="ExternalInput")
    ad_wT = nc.dram_tensor("ad_wT", [EDIM, EDIM], BF16, kind="ExternalInput")
    lin1T = nc.dram_tensor("lin1T", [EDIM, EDIM], BF16, kind="ExternalInput")
    lin2T = nc.dram_tensor("lin2T", [EDIM, EDIM], BF16, kind="ExternalInput")
    cls_wT = nc.dram_tensor("cls_wT", [EDIM, INUM], BF16, kind="ExternalInput")
    v0T16 = nc.dram_tensor("v0T16", [EDIM, CNUM], BF16, kind="ExternalInput")
    lin_b = nc.dram_tensor("lin_b", [EDIM], F32, kind="ExternalInput")
    er_b = nc.dram_tensor("er_b", [EDIM], F32, kind="ExternalInput")
    ad_b = nc.dram_tensor("ad_b", [EDIM], F32, kind="ExternalInput")
    cls_b16 = nc.dram_tensor("cls_b16", [1, INUM], BF16, kind="ExternalInput")
    out = nc.dram_tensor("out", [BT, INUM], F32, kind="ExternalOutput")
    w_r = nc.dram_tensor("w_r", [BL, CNUM, S], BF16, kind="Internal")

    with tile.TileContext(nc) as tc:
        with tc.tile_pool(name="singles", bufs=1) as sg:
            ones16 = sg.tile([1, 128], BF16, tag="ones16")
            nc.vector.memset(ones16[:], 1.0)
            ones50 = sg.tile([CNUM, 1], BF16, tag="ones50")
            nc.vector.memset(ones50[:], 1.0)

            A_w_sb = sg.tile([KC, IK, EDIM], BF16, tag="A_w_sb")
            B_w_sb = sg.tile([KC, DK, EDIM], BF16, tag="B_w_sb")
            kmat_sb = sg.tile([EDIM, CNUM], BF16, tag="kmat_sb")
            er_w_sb = sg.tile([EDIM, EDIM], BF16, tag="er_w_sb")
            ad_w_sb = sg.tile([EDIM, EDIM], BF16, tag="ad_w_sb")
            lin1_sb = sg.tile([EDIM, EDIM], BF16, tag="lin1_sb")
            lin2_sb = sg.tile([EDIM, EDIM], BF16, tag="lin2_sb")
            cls_w_sb = sg.tile([EDIM, INUM], BF16, tag="cls_w_sb")
            v0_sb = sg.tile([EDIM, CNUM], BF16, tag="v0_sb")
            lin_b_col = sg.tile([EDIM, 1], F32, tag="lin_b_col")
            er_b_col = sg.tile([EDIM, 1], F32, tag="er_b_col")
            ad_b_col = sg.tile([EDIM, 1], F32, tag="ad_b_col")
            cls_b_sb = sg.tile([1, INUM], BF16, tag="cls_b_sb")

            for k in range(IK):
                nc.sync.dma_start(A_w_sb[:, k, :],
                                  A_wT.ap()[k * KC:(k + 1) * KC, :])
            for k in range(DK):
                nc.sync.dma_start(B_w_sb[:, k, :],
                                  B_wT.ap()[k * KC:(k + 1) * KC, :])
            nc.sync.dma_start(kmat_sb[:], kmatT.ap())
            nc.sync.dma_start(er_w_sb[:], er_wT.ap())
            nc.sync.dma_start(ad_w_sb[:], ad_wT.ap())
            nc.sync.dma_start(lin1_sb[:], lin1T.ap())
            nc.sync.dma_start(lin2_sb[:], lin2T.ap())
            nc.sync.dma_start(cls_w_sb[:], cls_wT.ap())
            nc.sync.dma_start(v0_sb[:], v0T16.ap())
            nc.sync.dma_start(lin_b_col[:], lin_b.ap()[:, None])
            nc.sync.dma_start(er_b_col[:], er_b.ap()[:, None])
            nc.sync.dma_start(ad_b_col[:], ad_b.ap()[:, None])
            nc.sync.dma_start(cls_b_sb[:], cls_b16.ap())

            # persistent activations
            itm16 = sg.tile([EDIM, BT], BF16, tag="itm16")
            e16 = sg.tile([EDIM, BL, S], BF16, tag="e16")
            a16 = sg.tile([EDIM, BL, S], BF16, tag="a16")

            # ---- phase 1: full-width loads, then bt-chunks of 400;
            # softmax + w_r store per chunk so the scan's Wbc DMAs can
            # start as early as possible ----
            with tc.tile_pool(name="p1sb", bufs=1) as p1, \
                 tc.tile_pool(name="p1w", bufs=2) as p1w, \
                 tc.tile_pool(name="p1ps", bufs=2, space="PSUM") as p1p, \
                 tc.tile_pool(name="p1pss", bufs=1, space="PSUM") as p1ps:
                it_sb = p1.tile([KC, IK, BT], BF16, tag="it_sb")
                for k in range(IK):
                    nc.sync.dma_start(it_sb[:, k, :],
                                      itemT.ap()[k * KC:(k + 1) * KC, :])
                in_sb = p1.tile([KC, DK, BT], BF16, tag="in_sb")
                for k in range(DK):
                    nc.sync.dma_start(in_sb[:, k, :],
                                      interT.ap()[k * KC:(k + 1) * KC, :])

                for q in range(4):
                    c0 = q * 400
                    cols = slice(c0, c0 + 400)
                    ps_itm = p1p.tile([EDIM, 400], F32, tag="ps_itm")
                    for k in range(IK):
                        nc.tensor.matmul(ps_itm[:], A_w_sb[:, k, :],
                                         it_sb[:, k, cols],
                                         start=(k == 0), stop=(k == IK - 1))
                    nc.scalar.copy(itm16[:, cols], ps_itm[:])

                    # softmax for this chunk
                    ps_l = p1ps.tile([CNUM, 400], F32, tag="ps_l")
                    nc.tensor.matmul(ps_l[:], kmat_sb[:], itm16[:, cols],
                                     start=True, stop=True)
                    E_j = p1w.tile([CNUM, 400], BF16, tag="E_j")
                    nc.scalar.activation(E_j[:], ps_l[:], AF.Exp)
                    ps_z = p1ps.tile([1, 400], F32, tag="ps_z")
                    nc.tensor.matmul(ps_z[:], ones50[:], E_j[:],
                                     start=True, stop=True)
                    zr = p1w.tile([1, 400], F32, tag="zr")
                    nc.vector.reciprocal(zr[:], ps_z[:])
                    zr16 = p1w.tile([1, 400], BF16, tag="zr16")
                    nc.vector.tensor_copy(zr16[:], zr[:])
                    ps_zb = p1ps.tile([CNUM, 400], F32, tag="ps_zb")
                    nc.tensor.matmul(ps_zb[:], ones16[:1, :CNUM], zr16[:],
                                     start=True, stop=True)
                    w_j = p1w.tile([CNUM, 400], BF16, tag="w_j")
                    nc.vector.scalar_tensor_tensor(
                        out=w_j[:], in0=E_j[:], scalar=1.0,
                        in1=ps_zb[:], op0=OP.mult, op1=OP.mult)
                    nc.sync.dma_start(w_r.ap()[2 * q], w_j[:, 0:S])
                    nc.sync.dma_start(w_r.ap()[2 * q + 1], w_j[:, S:2 * S])

                    ps_itr = p1p.tile([EDIM, 400], F32, tag="ps_itr")
                    for k in range(DK):
                        nc.tensor.matmul(ps_itr[:], B_w_sb[:, k, :],
                                         in_sb[:, k, cols],
                                         start=(k == 0), stop=(k == DK - 1))
                    itr_j = p1w.tile([EDIM, 400], BF16, tag="itr_j")
                    nc.scalar.copy(itr_j[:], ps_itr[:])

                    j2 = q * 2
                    ps_e = p1ps.tile([EDIM, 400], F32, tag="ps_ea")
                    nc.tensor.matmul(ps_e[:], er_w_sb[:], itr_j[:],
                                     start=True, stop=True)
                    nc.scalar.activation(e16[:, j2:j2 + 2, :], ps_e[:],
                                         AF.Sigmoid, bias=er_b_col[:], scale=1.0)
                    ps_a = p1ps.tile([EDIM, 400], F32, tag="ps_ea")
                    nc.tensor.matmul(ps_a[:], ad_w_sb[:], itr_j[:],
                                     start=True, stop=True)
                    nc.scalar.activation(a16[:, j2:j2 + 2, :], ps_a[:],
                                         AF.Tanh, bias=ad_b_col[:], scale=1.0)

            # ---- phase 2: bulk scan per b ----
            with tc.tile_pool(name="scW", bufs=2) as scW, \
                 tc.tile_pool(name="scFM", bufs=1) as scFM, \
                 tc.tile_pool(name="scA", bufs=2) as scA, \
                 tc.tile_pool(name="scV", bufs=1) as scV, \
                 tc.tile_pool(name="scX", bufs=1) as scX, \
                 tc.tile_pool(name="hps_pool", bufs=2, space="PSUM") as hpsp, \
                 tc.tile_pool(name="clsps", bufs=2, space="PSUM") as clsp, \
                 tc.tile_pool(name="p3sb", bufs=2) as p3:
                hps = None
                XP = None
                for b in range(BL):
                    # W broadcast across partitions: one stride-0 DMA per b
                    Wbc = scW.tile([EDIM, CNUM, S], BF16, tag="Wbc")
                    w_flat = w_r.ap()[b:b + 1].rearrange("a c t -> a (c t)")
                    nc.sync.dma_start(
                        Wbc[:].rearrange("p c t -> p (c t)"),
                        w_flat.to_broadcast([EDIM, CT]))

                    e_bv = e16[:, b:b + 1, :].to_broadcast([EDIM, CNUM, S])
                    a_bv = a16[:, b:b + 1, :].to_broadcast([EDIM, CNUM, S])

                    A = scA.tile([EDIM, CNUM, S], BF16, tag="A")
                    _eng(nc, A_ENG[b]).tensor_tensor(out=A[:], in0=Wbc[:],
                                                     in1=a_bv, op=OP.mult)
                    FM = scFM.tile([EDIM, CNUM, S], FP16, tag="FM")
                    _eng(nc, FM_ENG[b]).tensor_tensor(out=FM[:], in0=Wbc[:],
                                                      in1=e_bv, op=OP.mult)
                    nc.vector.tensor_scalar(out=FM[:], in0=FM[:],
                                            scalar1=-1.0, scalar2=1.0,
                                            op0=OP.mult, op1=OP.add)
                    # t=0 fixup: A0 += M0*V0 ; M0 = 0
                    t0 = p3.tile([EDIM, CNUM, 1], F32, tag="t0")
                    nc.vector.scalar_tensor_tensor(
                        out=t0[:], in0=FM[:, :, 0:1], scalar=1.0,
                        in1=v0_sb[:, :, None], op0=OP.mult, op1=OP.mult)
                    nc.vector.tensor_tensor(out=A[:, :, 0:1], in0=A[:, :, 0:1],
                                            in1=t0[:], op=OP.add)
                    nc.vector.tensor_scalar(out=FM[:, :, 0:1], in0=FM[:, :, 0:1],
                                            scalar1=0.0, scalar2=None,
                                            op0=OP.mult)

                    V = scV.tile([EDIM, CNUM, S], BF16, tag="V")
                    nc.vector.tensor_tensor_scan(
                        out=V[:].rearrange("p c t -> p (c t)"),
                        data0=FM[:].rearrange("p c t -> p (c t)"),
                        data1=A[:].rearrange("p c t -> p (c t)"),
                        initial=0.0, op0=OP.mult, op1=OP.add)

                    half = b % 2
                    if half == 0:
                        XP = scX.tile([EDIM, CNUM, 2, S], BF16, tag="XP")
                    _eng(nc, X_ENG[b]).tensor_tensor(
                        out=XP[:, :, half, 1:], in0=Wbc[:, :, 1:],
                        in1=V[:, :, 0:S - 1], op=OP.mult)
                    nc.vector.tensor_tensor(out=XP[:, :, half, 0:1],
                                            in0=Wbc[:, :, 0:1],
                                            in1=v0_sb[:, :, None], op=OP.mult)

                    # ---- phase 3 per completed pair ----
                    if half == 1:
                        # r-projection for the pair:
                        # hps[:, (half,t)] = sum_c lin1 @ XP[:,c,:,:] + lin2 @ itm
                        hps = hpsp.tile([EDIM, 2 * S], F32, tag="hps")
                        for c in range(CNUM):
                            nc.tensor.matmul(hps[:], lin1_sb[:],
                                             XP[:, c, :, :],
                                             start=(c == 0), stop=False)
                        nc.tensor.matmul(hps[:], lin2_sb[:],
                                         itm16[:, (b - 1) * S:(b + 1) * S],
                                         start=False, stop=True)
                        c0 = (b - 1) * S
                        h16 = p3.tile([EDIM, 2 * S], BF16, tag="h16")
                        nc.scalar.activation(h16[:], hps[:], AF.Tanh,
                                             bias=lin_b_col[:], scale=1.0)
                        for (s0, sw) in ((0, 128), (128, 72),
                                         (200, 128), (328, 72)):
                            ot = p3.tile([128, INUM], F32, tag="ot")
                            for hf in range(2):
                                hc = slice(hf * 500, hf * 500 + 500)
                                ps_o = clsp.tile([128, 500], F32, tag="ps_o")
                                nc.tensor.matmul(ps_o[:sw], h16[:, s0:s0 + sw],
                                                 cls_w_sb[:, hc],
                                                 start=True, stop=False)
                                nc.tensor.matmul(ps_o[:sw], ones16[:1, :sw],
                                                 cls_b_sb[:, hc],
                                                 start=False, stop=True)
                                nc.scalar.activation(ot[:sw, hc], ps_o[:sw],
                                                     AF.Sigmoid)
                            nc.sync.dma_start(
                                out.ap()[c0 + s0: c0 + s0 + sw], ot[:sw])

    nc.compile()
    return nc


def kernel(**inputs):
    global LAST_RESULT
    if "nc" not in _NC_CACHE:
        _NC_CACHE["nc"] = _build()
    nc = _NC_CACHE["nc"]

    bf = ml_dtypes.bfloat16
    f32 = {k: np.asarray(inputs[k], dtype=np.float32) for k in inputs}
    lin_w = f32["lin_w"]
    shared = {
        "A_wT": np.ascontiguousarray(f32["A_w"].T).astype(bf),
        "B_wT": np.ascontiguousarray(f32["B_w"].T).astype(bf),
        "kmatT": np.ascontiguousarray(f32["kmat"].T).astype(bf),
        "er_wT": np.ascontiguousarray(f32["er_w"].T).astype(bf),
        "ad_wT": np.ascontiguousarray(f32["ad_w"].T).astype(bf),
        "lin1T": np.ascontiguousarray(lin_w[:, :EDIM].T).astype(bf),
        "lin2T": np.ascontiguousarray(lin_w[:, EDIM:].T).astype(bf),
        "cls_wT": np.ascontiguousarray(f32["cls_w"].T).astype(bf),
        "v0T16": np.ascontiguousarray(f32["vmat0"].T).astype(bf),
        "lin_b": f32["lin_b"],
        "er_b": f32["er_b"],
        "ad_b": f32["ad_b"],
        "cls_b16": np.ascontiguousarray(f32["cls_b"][None, :]).astype(bf),
    }
    item = f32["item"]
    inter = f32["interaction"]

    in_maps = []
    for c in range(NCORES):
        m = dict(shared)
        m["itemT"] = np.ascontiguousarray(
            item[c * BL:(c + 1) * BL].reshape(BT, INUM).T).astype(bf)
        m["interT"] = np.ascontiguousarray(
            inter[c * BL:(c + 1) * BL].reshape(BT, IN_DIM).T).astype(bf)
        in_maps.append(m)

    res = run_bass_kernel_spmd(nc, in_maps, core_ids=list(range(NCORES)))
    LAST_RESULT = res
    outs = [res.results[c]["out"].reshape(BL, S, INUM) for c in range(NCORES)]
    return np.concatenate(outs, axis=0)
